# revision 18
# baseline (speedup 1.0000x reference)
"""Chamfer distance (pytorch3d defaults) on 8 Trainium2 NeuronCores.

Problem: gts_X, pred_X: [4, 8192, 3] fp32. loss = mean_b mean_n min_p d(x_bn, y_bp)
                                              + mean_b mean_p min_n d(x_bn, y_bp),
d = squared euclidean distance. gts_normals is unused (reference default path).

Sharding: 8 independent tasks = 4 batches x 2 directions, one per core.
Each core computes per-query min_r d(Q_q, R_r) for its (Q, R) pair of
8192-point clouds; the host sums, guards, and averages.

Device algorithm per core (v3):
- Both clouds are sorted by the z coordinate on the host. Each query
  super-block (4 row blocks of 128 sorted queries) only scans a WINDOW of
  WIN_TILES=9 ref col-tiles (4608 sorted refs) centered on its rank range.
  A query's true nearest neighbor can only be outside the window if the
  squared z-gap to the window edge is smaller than the found min; the host
  verifies that condition per query and recomputes the (rare/none) escapes
  exactly in numpy, so the result is exact for any input.
- d[q, r] = |Q|^2 + |R|^2 - 2 Q.R via ONE K=16 bf16 matmul per (128q x 512r)
  tile using an exact hi/lo bf16 split (bf16 products are exact in fp32, PSUM
  accumulates fp32 => ~fp32 precision).
- Matmuls are packed 4x with tile_position row groups.
- Min-reduction: DIRECT_SET col-tiles are min-reduced straight from PSUM by
  the DVE (1x mode); the rest are ACT-copied PSUM->SBUF with a bf16 downcast
  and folded by a DVE tensor_tensor min tree in 2x bf16 mode.
"""

import sys

sys.path.insert(0, "/opt/trn_rl_repo")

import numpy as np
import ml_dtypes

import concourse.bacc as bacc
import concourse.mybir as mybir
from concourse.tile import TileContext
from concourse.bass_utils import run_bass_kernel_spmd

BF16 = ml_dtypes.bfloat16

B = 4
N = 8192
K = 16  # contraction rows after hi/lo split
MBLK = 128  # queries per row block (PSUM partitions)
NBLK = 512  # refs per matmul (one PSUM bank of fp32)
NMB = N // MBLK  # 64 row blocks
NNB = N // NBLK  # 16 col tiles
SB = NMB // 4  # 16 super-blocks of 4 row blocks

WIN_TILES = 9  # ref col-tiles scanned per super-block
# within-window positions reduced directly from PSUM by the DVE (interleaved
# with ACT-copied positions so the PSUM-slot release chain alternates engines)
DIRECT_POS = (0, 4, 8)
ACT_POS = tuple(t for t in range(WIN_TILES) if t not in DIRECT_POS)
DIRECT_COLS = len(DIRECT_POS)
ACT_COLS = len(ACT_POS)
HALF = ACT_COLS // 2  # two half-trees of HALF cols each

LAST_RESULTS = None  # BassKernelResults of the most recent run (for test.py)


def _win_start(s):
    """First ref col-tile of super-block s's window (rank-based, static)."""
    return min(max(s - WIN_TILES // 2, 0), NNB - WIN_TILES)


def _tt_min(nc, out, a, b):
    nc.vector.tensor_tensor(out, a, b, op=mybir.AluOpType.min)


def _half_tree(nc, work_pool, bfb, part_col):
    """Fold bfb [128, 4, HALF*512] bf16 down to part_col [128, 4, 1] fp32
    via 2x-mode TT mins + one short 1x reduce. szX are per-partition free
    element counts."""
    sz1 = HALF * 512 // 2  # per-block run after level 1
    sz2 = sz1 // 2
    sz3 = sz2 // 2
    sz4 = sz3 // 2
    t1 = work_pool.tile([MBLK, 4, sz1], mybir.dt.bfloat16, tag="t1")
    t2 = work_pool.tile([MBLK, 4, sz2], mybir.dt.bfloat16, tag="t2")
    t3 = work_pool.tile([MBLK, 4, sz3], mybir.dt.bfloat16, tag="t3")
    t4 = work_pool.tile([MBLK, 4, sz4], mybir.dt.bfloat16, tag="t4")
    _tt_min(nc, t1[:], bfb[:, :, 0:sz1], bfb[:, :, sz1 : 2 * sz1])
    _tt_min(nc, t2[:], t1[:, :, 0:sz2], t1[:, :, sz2 : 2 * sz2])
    _tt_min(nc, t3[:], t2[:, :, 0:sz3], t2[:, :, sz3 : 2 * sz3])
    _tt_min(nc, t4[:], t3[:, :, 0:sz4], t3[:, :, sz4 : 2 * sz4])
    nc.vector.tensor_reduce(
        part_col, t4[:], axis=mybir.AxisListType.X, op=mybir.AluOpType.min
    )


def _build_bass():
    nc = bacc.Bacc("TRN2")
    lhs = nc.dram_tensor("lhs", [K, N], mybir.dt.bfloat16, kind="ExternalInput")
    rhs = nc.dram_tensor("rhs", [K, N], mybir.dt.bfloat16, kind="ExternalInput")
    out = nc.dram_tensor("out", [MBLK, NMB], mybir.dt.float32, kind="ExternalOutput")

    with TileContext(nc) as tc:
        with (
            tc.tile_pool(name="data", bufs=1) as data_pool,
            tc.tile_pool(name="work", bufs=3) as work_pool,
            tc.tile_pool(name="ps", bufs=4, space="PSUM") as ps_pool,
        ):
            # operands replicated at partition offsets 0/32/64/96 so four
            # row-group-packed matmuls can run concurrently
            lhs_sb = data_pool.tile([128, N], mybir.dt.bfloat16)
            rhs_sb = data_pool.tile([128, N], mybir.dt.bfloat16)
            for g in range(4):
                nc.sync.dma_start(lhs_sb[32 * g : 32 * g + K, :], lhs.ap())
                nc.sync.dma_start(rhs_sb[32 * g : 32 * g + K, :], rhs.ap())

            blockmins = data_pool.tile([MBLK, NMB], mybir.dt.float32)

            for s in range(SB):
                w0 = _win_start(s)
                part = work_pool.tile(
                    [MBLK, 4, DIRECT_COLS + 2], mybir.dt.float32, tag="part"
                )
                bfb0 = work_pool.tile(
                    [MBLK, 4, HALF * 512], mybir.dt.bfloat16, tag="bfb0"
                )
                bfb1 = work_pool.tile(
                    [MBLK, 4, HALF * 512], mybir.dt.bfloat16, tag="bfb1"
                )
                for t in range(WIN_TILES):
                    n = w0 + t
                    # two 2-bank PSUM tiles per col (blocks 0-1 and 2-3) so
                    # the pool has 4 slots in flight and consumers split into
                    # shorter units -> less head-of-line blocking
                    ps_a = ps_pool.tile([MBLK, 2, NBLK], mybir.dt.float32, tag="ps")
                    ps_b = ps_pool.tile([MBLK, 2, NBLK], mybir.dt.float32, tag="ps")
                    pshalves = [ps_a, ps_b]
                    for j in range(4):
                        m = 4 * s + j
                        nc.tensor.matmul(
                            pshalves[j // 2][:, j % 2, :],
                            lhs_sb[32 * j : 32 * j + K, m * MBLK : (m + 1) * MBLK],
                            rhs_sb[32 * j : 32 * j + K, n * NBLK : (n + 1) * NBLK],
                            start=True,
                            stop=True,
                            tile_position=(32 * j, 0),
                        )
                    if t in DIRECT_POS:
                        for h in range(2):
                            nc.vector.tensor_reduce(
                                part[:, 2 * h : 2 * h + 2, DIRECT_POS.index(t)],
                                pshalves[h][:],
                                axis=mybir.AxisListType.X,
                                op=mybir.AluOpType.min,
                            )
                    else:
                        c = ACT_POS.index(t)
                        dst = bfb0 if c < HALF else bfb1
                        co = (c % HALF) * 512
                        for h in range(2):
                            nc.scalar.copy(
                                dst[:, 2 * h : 2 * h + 2, co : co + 512],
                                pshalves[h][:],
                            )
                    if t == ACT_POS[HALF - 1]:
                        _half_tree(nc, work_pool, bfb0, part[:, :, DIRECT_COLS])
                    elif t == ACT_POS[-1]:
                        _half_tree(nc, work_pool, bfb1, part[:, :, DIRECT_COLS + 1])
                nc.vector.tensor_reduce(
                    blockmins[:, 4 * s : 4 * s + 4],
                    part[:],
                    axis=mybir.AxisListType.X,
                    op=mybir.AluOpType.min,
                )

            nc.sync.dma_start(out.ap(), blockmins[:])
    return nc


def _split_bf16(v):
    """v (fp32) ~= hi + lo with both bf16; residual is O(2^-18 |v|)."""
    hi = v.astype(BF16)
    lo = (v - hi.astype(np.float32)).astype(BF16)
    return hi, lo


def _prep_core_inputs(Q, R):
    """Build the K=16 lhsT (queries) and rhs (refs) bf16 matrices so that
    lhsT.T @ rhs accumulated in fp32 equals |Q|^2 + |R|^2 - 2 Q.R."""
    Qh, Ql = _split_bf16(Q)  # [N, 3]
    Rh, Rl = _split_bf16(-2.0 * R)  # [N, 3]
    nQh, nQl = _split_bf16((Q * Q).sum(axis=1))  # [N]
    nRh, nRl = _split_bf16((R * R).sum(axis=1))  # [N]
    one = np.ones(N, dtype=BF16)

    L = np.empty([K, N], dtype=BF16)
    L[0:3] = Qh.T
    L[3:6] = Qh.T
    L[6:9] = Ql.T
    L[9:12] = Ql.T
    L[12] = nQh
    L[13] = nQl
    L[14] = one
    L[15] = one

    Rm = np.empty([K, N], dtype=BF16)
    Rm[0:3] = Rh.T
    Rm[3:6] = Rl.T
    Rm[6:9] = Rh.T
    Rm[9:12] = Rl.T
    Rm[12] = one
    Rm[13] = one
    Rm[14] = nRh
    Rm[15] = nRl
    return L, Rm


def _try_axon_reset():
    """The axon-tunneled device sporadically wedges (NRT_EXEC_UNIT_UNRECOVERABLE);
    axon_reset() recovers it."""
    try:
        import ctypes

        import jax

        jax.devices()
        lib = ctypes.CDLL("/opt/axon/libaxon_pjrt.so")
        lib.axon_reset.restype = ctypes.c_int64
        lib.axon_reset()
    except Exception:
        pass


def _task_pairs(gts_X, pred_X):
    for b in range(B):
        yield gts_X[b], pred_X[b]  # each gts point -> nearest pred
        yield pred_X[b], gts_X[b]  # each pred point -> nearest gts


def kernel(gts_X, pred_X, gts_normals=None, **_ignored):
    global LAST_RESULTS
    gts_X = np.asarray(gts_X, dtype=np.float32)
    pred_X = np.asarray(pred_X, dtype=np.float32)
    assert gts_X.shape == (B, N, 3) and pred_X.shape == (B, N, 3)

    in_maps = []
    sorted_pairs = []
    for Qr, Rr in _task_pairs(gts_X, pred_X):
        Qs = np.ascontiguousarray(Qr[np.argsort(Qr[:, 2], kind="stable")])
        Rs = np.ascontiguousarray(Rr[np.argsort(Rr[:, 2], kind="stable")])
        sorted_pairs.append((Qs, Rs))
        L, Rm = _prep_core_inputs(Qs, Rs)
        in_maps.append({"lhs": L, "rhs": Rm})

    nc = _build_bass()
    nc.finalize()
    res = None
    for attempt in range(3):
        try:
            res = run_bass_kernel_spmd(nc, in_maps, core_ids=list(range(8)))
            break
        except Exception:
            if attempt == 2:
                raise
            _try_axon_reset()
    LAST_RESULTS = res

    total = 0.0
    for (Qs, Rs), r in zip(sorted_pairs, res.results):
        mins = r["out"].astype(np.float64)  # [128, 64]; query rank = m*128 + p
        mins = mins.T.reshape(-1)  # rank-ordered per-query windowed mins
        # exactness guard: the true NN can only lie outside the window if the
        # squared z-gap to the window edge is below the windowed min
        s_idx = np.arange(N) // (4 * MBLK)
        w0 = np.array([_win_start(int(s)) for s in range(SB)])[s_idx]
        lo = w0 * NBLK  # first ref rank in window
        hi = lo + WIN_TILES * NBLK  # one past last
        zq = Qs[:, 2].astype(np.float64)
        zr = Rs[:, 2].astype(np.float64)
        gap_lo = np.where(lo > 0, zq - zr[np.maximum(lo - 1, 0)], np.inf)
        gap_hi = np.where(hi < N, zr[np.minimum(hi, N - 1)] - zq, np.inf)
        guard = np.minimum(gap_lo, gap_hi) ** 2
        bad = np.nonzero(mins > guard)[0]
        if len(bad):
            Qb = Qs[bad].astype(np.float64)
            d = ((Qb[:, None, :] - Rs[None, :, :].astype(np.float64)) ** 2).sum(-1)
            mins[bad] = d.min(axis=1)
        total += mins.sum()

    loss = total / (B * N)
    return np.asarray(loss, dtype=np.float32)


# revision 19
# speedup vs baseline: 1.3582x; 1.3582x over previous
"""Chamfer distance (pytorch3d defaults) on 8 Trainium2 NeuronCores.

Problem: gts_X, pred_X: [4, 8192, 3] fp32. loss = mean_b mean_n min_p d(x_bn, y_bp)
                                              + mean_b mean_p min_n d(x_bn, y_bp),
d = squared euclidean distance. gts_normals is unused (reference default path).

Sharding: 8 independent tasks = 4 batches x 2 directions, one per core.
Each core computes per-query min_r d(Q_q, R_r) for its (Q, R) pair of
8192-point clouds; the host sums, guards, and averages.

Device algorithm per core (v3):
- Both clouds are sorted by the z coordinate on the host. Each query
  super-block (4 row blocks of 128 sorted queries) only scans a WINDOW of
  WIN_TILES=9 ref col-tiles (4608 sorted refs) centered on its rank range.
  A query's true nearest neighbor can only be outside the window if the
  squared z-gap to the window edge is smaller than the found min; the host
  verifies that condition per query and recomputes the (rare/none) escapes
  exactly in numpy, so the result is exact for any input.
- d[q, r] = |Q|^2 + |R|^2 - 2 Q.R via ONE K=16 bf16 matmul per (128q x 512r)
  tile using an exact hi/lo bf16 split (bf16 products are exact in fp32, PSUM
  accumulates fp32 => ~fp32 precision).
- Matmuls are packed 4x with tile_position row groups.
- Min-reduction: DIRECT_SET col-tiles are min-reduced straight from PSUM by
  the DVE (1x mode); the rest are ACT-copied PSUM->SBUF with a bf16 downcast
  and folded by a DVE tensor_tensor min tree in 2x bf16 mode.
"""

import sys

sys.path.insert(0, "/opt/trn_rl_repo")

import numpy as np
import ml_dtypes

import concourse.bacc as bacc
import concourse.mybir as mybir
from concourse.tile import TileContext
from concourse.bass_utils import run_bass_kernel_spmd

BF16 = ml_dtypes.bfloat16

B = 4
N = 8192
K = 16  # contraction rows after hi/lo split
MBLK = 128  # queries per row block (PSUM partitions)
NBLK = 512  # refs per matmul (one PSUM bank of fp32)
NMB = N // MBLK  # 64 row blocks
NNB = N // NBLK  # 16 col tiles
SB = NMB // 4  # 16 super-blocks of 4 row blocks

WIN_TILES = 8  # ref col-tiles scanned per super-block
# within-window positions reduced directly from PSUM by the DVE (interleaved
# with ACT-copied positions so the PSUM-slot release chain alternates engines)
DIRECT_POS = (0, 4)
ACT_POS = tuple(t for t in range(WIN_TILES) if t not in DIRECT_POS)
DIRECT_COLS = len(DIRECT_POS)
ACT_COLS = len(ACT_POS)
HALF = ACT_COLS // 2  # two half-trees of HALF cols each

LAST_RESULTS = None  # BassKernelResults of the most recent run (for test.py)


def _win_start(s):
    """First ref col-tile of super-block s's window (rank-based, static)."""
    return min(max(s - WIN_TILES // 2, 0), NNB - WIN_TILES)


def _tt_min(nc, out, a, b):
    nc.vector.tensor_tensor(out, a, b, op=mybir.AluOpType.min)


def _half_tree(nc, work_pool, bfb, part_col):
    """Fold bfb [128, 4, HALF*512] bf16 down to part_col [128, 4, 1] fp32
    via 2x-mode TT mins + one short 1x reduce. szX are per-partition free
    element counts."""
    sz1 = HALF * 512 // 2  # per-block run after level 1
    sz2 = sz1 // 2
    sz3 = sz2 // 2
    sz4 = sz3 // 2
    t1 = work_pool.tile([MBLK, 4, sz1], mybir.dt.bfloat16, tag="t1")
    t2 = work_pool.tile([MBLK, 4, sz2], mybir.dt.bfloat16, tag="t2")
    t3 = work_pool.tile([MBLK, 4, sz3], mybir.dt.bfloat16, tag="t3")
    t4 = work_pool.tile([MBLK, 4, sz4], mybir.dt.bfloat16, tag="t4")
    _tt_min(nc, t1[:], bfb[:, :, 0:sz1], bfb[:, :, sz1 : 2 * sz1])
    _tt_min(nc, t2[:], t1[:, :, 0:sz2], t1[:, :, sz2 : 2 * sz2])
    _tt_min(nc, t3[:], t2[:, :, 0:sz3], t2[:, :, sz3 : 2 * sz3])
    _tt_min(nc, t4[:], t3[:, :, 0:sz4], t3[:, :, sz4 : 2 * sz4])
    nc.vector.tensor_reduce(
        part_col, t4[:], axis=mybir.AxisListType.X, op=mybir.AluOpType.min
    )


def _build_bass():
    nc = bacc.Bacc("TRN2")
    lhs = nc.dram_tensor("lhs", [K, N], mybir.dt.bfloat16, kind="ExternalInput")
    rhs = nc.dram_tensor("rhs", [K, N], mybir.dt.bfloat16, kind="ExternalInput")
    out = nc.dram_tensor("out", [MBLK, NMB], mybir.dt.float32, kind="ExternalOutput")

    with TileContext(nc) as tc:
        with (
            tc.tile_pool(name="data", bufs=1) as data_pool,
            tc.tile_pool(name="work", bufs=3) as work_pool,
            tc.tile_pool(name="ps", bufs=4, space="PSUM") as ps_pool,
        ):
            # operands replicated at partition offsets 0/32/64/96 so four
            # row-group-packed matmuls can run concurrently
            lhs_sb = data_pool.tile([128, N], mybir.dt.bfloat16)
            rhs_sb = data_pool.tile([128, N], mybir.dt.bfloat16)
            for g in range(4):
                nc.sync.dma_start(lhs_sb[32 * g : 32 * g + K, :], lhs.ap())
                nc.sync.dma_start(rhs_sb[32 * g : 32 * g + K, :], rhs.ap())

            blockmins = data_pool.tile([MBLK, NMB], mybir.dt.float32)

            for s in range(SB):
                w0 = _win_start(s)
                part = work_pool.tile(
                    [MBLK, 4, DIRECT_COLS + 2], mybir.dt.float32, tag="part"
                )
                bfb0 = work_pool.tile(
                    [MBLK, 4, HALF * 512], mybir.dt.bfloat16, tag="bfb0"
                )
                bfb1 = work_pool.tile(
                    [MBLK, 4, HALF * 512], mybir.dt.bfloat16, tag="bfb1"
                )
                for t in range(WIN_TILES):
                    n = w0 + t
                    # two 2-bank PSUM tiles per col (blocks 0-1 and 2-3) so
                    # the pool has 4 slots in flight and consumers split into
                    # shorter units -> less head-of-line blocking
                    ps_a = ps_pool.tile([MBLK, 2, NBLK], mybir.dt.float32, tag="ps")
                    ps_b = ps_pool.tile([MBLK, 2, NBLK], mybir.dt.float32, tag="ps")
                    pshalves = [ps_a, ps_b]
                    for j in range(4):
                        m = 4 * s + j
                        nc.tensor.matmul(
                            pshalves[j // 2][:, j % 2, :],
                            lhs_sb[32 * j : 32 * j + K, m * MBLK : (m + 1) * MBLK],
                            rhs_sb[32 * j : 32 * j + K, n * NBLK : (n + 1) * NBLK],
                            start=True,
                            stop=True,
                            tile_position=(32 * j, 0),
                        )
                    if t in DIRECT_POS:
                        for h in range(2):
                            nc.vector.tensor_reduce(
                                part[:, 2 * h : 2 * h + 2, DIRECT_POS.index(t)],
                                pshalves[h][:],
                                axis=mybir.AxisListType.X,
                                op=mybir.AluOpType.min,
                            )
                    else:
                        c = ACT_POS.index(t)
                        dst = bfb0 if c < HALF else bfb1
                        co = (c % HALF) * 512
                        for h in range(2):
                            nc.scalar.copy(
                                dst[:, 2 * h : 2 * h + 2, co : co + 512],
                                pshalves[h][:],
                            )
                    if t == ACT_POS[HALF - 1]:
                        _half_tree(nc, work_pool, bfb0, part[:, :, DIRECT_COLS])
                    elif t == ACT_POS[-1]:
                        _half_tree(nc, work_pool, bfb1, part[:, :, DIRECT_COLS + 1])
                nc.vector.tensor_reduce(
                    blockmins[:, 4 * s : 4 * s + 4],
                    part[:],
                    axis=mybir.AxisListType.X,
                    op=mybir.AluOpType.min,
                )

            nc.sync.dma_start(out.ap(), blockmins[:])
    return nc


def _split_bf16(v):
    """v (fp32) ~= hi + lo with both bf16; residual is O(2^-18 |v|)."""
    hi = v.astype(BF16)
    lo = (v - hi.astype(np.float32)).astype(BF16)
    return hi, lo


def _prep_core_inputs(Q, R):
    """Build the K=16 lhsT (queries) and rhs (refs) bf16 matrices so that
    lhsT.T @ rhs accumulated in fp32 equals |Q|^2 + |R|^2 - 2 Q.R."""
    Qh, Ql = _split_bf16(Q)  # [N, 3]
    Rh, Rl = _split_bf16(-2.0 * R)  # [N, 3]
    nQh, nQl = _split_bf16((Q * Q).sum(axis=1))  # [N]
    nRh, nRl = _split_bf16((R * R).sum(axis=1))  # [N]
    one = np.ones(N, dtype=BF16)

    L = np.empty([K, N], dtype=BF16)
    L[0:3] = Qh.T
    L[3:6] = Qh.T
    L[6:9] = Ql.T
    L[9:12] = Ql.T
    L[12] = nQh
    L[13] = nQl
    L[14] = one
    L[15] = one

    Rm = np.empty([K, N], dtype=BF16)
    Rm[0:3] = Rh.T
    Rm[3:6] = Rl.T
    Rm[6:9] = Rh.T
    Rm[9:12] = Rl.T
    Rm[12] = one
    Rm[13] = one
    Rm[14] = nRh
    Rm[15] = nRl
    return L, Rm


def _try_axon_reset():
    """The axon-tunneled device sporadically wedges (NRT_EXEC_UNIT_UNRECOVERABLE);
    axon_reset() recovers it."""
    try:
        import ctypes

        import jax

        jax.devices()
        lib = ctypes.CDLL("/opt/axon/libaxon_pjrt.so")
        lib.axon_reset.restype = ctypes.c_int64
        lib.axon_reset()
    except Exception:
        pass


def _task_pairs(gts_X, pred_X):
    for b in range(B):
        yield gts_X[b], pred_X[b]  # each gts point -> nearest pred
        yield pred_X[b], gts_X[b]  # each pred point -> nearest gts


def kernel(gts_X, pred_X, gts_normals=None, **_ignored):
    global LAST_RESULTS
    gts_X = np.asarray(gts_X, dtype=np.float32)
    pred_X = np.asarray(pred_X, dtype=np.float32)
    assert gts_X.shape == (B, N, 3) and pred_X.shape == (B, N, 3)

    in_maps = []
    sorted_pairs = []
    for Qr, Rr in _task_pairs(gts_X, pred_X):
        Qs = np.ascontiguousarray(Qr[np.argsort(Qr[:, 2], kind="stable")])
        Rs = np.ascontiguousarray(Rr[np.argsort(Rr[:, 2], kind="stable")])
        sorted_pairs.append((Qs, Rs))
        L, Rm = _prep_core_inputs(Qs, Rs)
        in_maps.append({"lhs": L, "rhs": Rm})

    nc = _build_bass()
    nc.finalize()
    res = None
    for attempt in range(3):
        try:
            res = run_bass_kernel_spmd(nc, in_maps, core_ids=list(range(8)))
            break
        except Exception:
            if attempt == 2:
                raise
            _try_axon_reset()
    LAST_RESULTS = res

    total = 0.0
    for (Qs, Rs), r in zip(sorted_pairs, res.results):
        mins = r["out"].astype(np.float64)  # [128, 64]; query rank = m*128 + p
        mins = mins.T.reshape(-1)  # rank-ordered per-query windowed mins
        # exactness guard: the true NN can only lie outside the window if the
        # squared z-gap to the window edge is below the windowed min
        s_idx = np.arange(N) // (4 * MBLK)
        w0 = np.array([_win_start(int(s)) for s in range(SB)])[s_idx]
        lo = w0 * NBLK  # first ref rank in window
        hi = lo + WIN_TILES * NBLK  # one past last
        zq = Qs[:, 2].astype(np.float64)
        zr = Rs[:, 2].astype(np.float64)
        gap_lo = np.where(lo > 0, zq - zr[np.maximum(lo - 1, 0)], np.inf)
        gap_hi = np.where(hi < N, zr[np.minimum(hi, N - 1)] - zq, np.inf)
        guard = np.minimum(gap_lo, gap_hi) ** 2
        bad = np.nonzero(mins > guard)[0]
        if len(bad):
            Qb = Qs[bad].astype(np.float64)
            d = ((Qb[:, None, :] - Rs[None, :, :].astype(np.float64)) ** 2).sum(-1)
            mins[bad] = d.min(axis=1)
        total += mins.sum()

    loss = total / (B * N)
    return np.asarray(loss, dtype=np.float32)


# revision 20
# speedup vs baseline: 1.6549x; 1.2185x over previous
"""Chamfer distance (pytorch3d defaults) on 8 Trainium2 NeuronCores.

Problem: gts_X, pred_X: [4, 8192, 3] fp32. loss = mean_b mean_n min_p d(x_bn, y_bp)
                                              + mean_b mean_p min_n d(x_bn, y_bp),
d = squared euclidean distance. gts_normals is unused (reference default path).

Sharding: 8 independent tasks = 4 batches x 2 directions, one per core.
Each core computes per-query min_r d(Q_q, R_r) for its (Q, R) pair of
8192-point clouds; the host sums, guards, and averages.

Device algorithm per core (v3):
- Both clouds are sorted by the z coordinate on the host. Each query
  super-block (4 row blocks of 128 sorted queries) only scans a WINDOW of
  WIN_TILES=9 ref col-tiles (4608 sorted refs) centered on its rank range.
  A query's true nearest neighbor can only be outside the window if the
  squared z-gap to the window edge is smaller than the found min; the host
  verifies that condition per query and recomputes the (rare/none) escapes
  exactly in numpy, so the result is exact for any input.
- d[q, r] = |Q|^2 + |R|^2 - 2 Q.R via ONE K=16 bf16 matmul per (128q x 512r)
  tile using an exact hi/lo bf16 split (bf16 products are exact in fp32, PSUM
  accumulates fp32 => ~fp32 precision).
- Matmuls are packed 4x with tile_position row groups.
- Min-reduction: DIRECT_SET col-tiles are min-reduced straight from PSUM by
  the DVE (1x mode); the rest are ACT-copied PSUM->SBUF with a bf16 downcast
  and folded by a DVE tensor_tensor min tree in 2x bf16 mode.
"""

import sys

sys.path.insert(0, "/opt/trn_rl_repo")

import numpy as np
import ml_dtypes

import concourse.bacc as bacc
import concourse.mybir as mybir
from concourse.tile import TileContext
from concourse.bass_utils import run_bass_kernel_spmd

BF16 = ml_dtypes.bfloat16

B = 4
N = 8192
K = 16  # contraction rows after hi/lo split
MBLK = 128  # queries per row block (PSUM partitions)
NBLK = 512  # refs per matmul (one PSUM bank of fp32)
NMB = N // MBLK  # 64 row blocks
NNB = N // NBLK  # 16 col tiles
SB = NMB // 4  # 16 super-blocks of 4 row blocks

WIN_TILES = 6  # ref col-tiles scanned per super-block
# within-window positions reduced directly from PSUM by the DVE (interleaved
# with ACT-copied positions so the PSUM-slot release chain alternates engines)
DIRECT_POS = (0, 3)
ACT_POS = tuple(t for t in range(WIN_TILES) if t not in DIRECT_POS)
DIRECT_COLS = len(DIRECT_POS)
ACT_COLS = len(ACT_POS)
HALF = ACT_COLS // 2  # two half-trees of HALF cols each

LAST_RESULTS = None  # BassKernelResults of the most recent run (for test.py)


def _win_start(s):
    """First ref col-tile of super-block s's window (rank-based, static)."""
    return min(max(s - WIN_TILES // 2, 0), NNB - WIN_TILES)


def _tt_min(nc, out, a, b):
    nc.vector.tensor_tensor(out, a, b, op=mybir.AluOpType.min)


def _half_tree(nc, work_pool, bfb, part_col):
    """Fold bfb [128, 4, HALF*512] bf16 down to part_col [128, 4, 1] fp32
    via 2x-mode TT mins + one short 1x reduce. szX are per-partition free
    element counts."""
    sz1 = HALF * 512 // 2  # per-block run after level 1
    sz2 = sz1 // 2
    sz3 = sz2 // 2
    sz4 = sz3 // 2
    t1 = work_pool.tile([MBLK, 4, sz1], mybir.dt.bfloat16, tag="t1")
    t2 = work_pool.tile([MBLK, 4, sz2], mybir.dt.bfloat16, tag="t2")
    t3 = work_pool.tile([MBLK, 4, sz3], mybir.dt.bfloat16, tag="t3")
    t4 = work_pool.tile([MBLK, 4, sz4], mybir.dt.bfloat16, tag="t4")
    _tt_min(nc, t1[:], bfb[:, :, 0:sz1], bfb[:, :, sz1 : 2 * sz1])
    _tt_min(nc, t2[:], t1[:, :, 0:sz2], t1[:, :, sz2 : 2 * sz2])
    _tt_min(nc, t3[:], t2[:, :, 0:sz3], t2[:, :, sz3 : 2 * sz3])
    _tt_min(nc, t4[:], t3[:, :, 0:sz4], t3[:, :, sz4 : 2 * sz4])
    nc.vector.tensor_reduce(
        part_col, t4[:], axis=mybir.AxisListType.X, op=mybir.AluOpType.min
    )


def _build_bass():
    nc = bacc.Bacc("TRN2")
    lhs = nc.dram_tensor("lhs", [K, N], mybir.dt.bfloat16, kind="ExternalInput")
    rhs = nc.dram_tensor("rhs", [K, N], mybir.dt.bfloat16, kind="ExternalInput")
    out = nc.dram_tensor("out", [MBLK, NMB], mybir.dt.float32, kind="ExternalOutput")

    with TileContext(nc) as tc:
        with (
            tc.tile_pool(name="data", bufs=1) as data_pool,
            tc.tile_pool(name="work", bufs=3) as work_pool,
            tc.tile_pool(name="ps", bufs=4, space="PSUM") as ps_pool,
        ):
            # operands replicated at partition offsets 0/32/64/96 so four
            # row-group-packed matmuls can run concurrently
            lhs_sb = data_pool.tile([128, N], mybir.dt.bfloat16)
            rhs_sb = data_pool.tile([128, N], mybir.dt.bfloat16)
            for g in range(4):
                nc.sync.dma_start(lhs_sb[32 * g : 32 * g + K, :], lhs.ap())
                nc.sync.dma_start(rhs_sb[32 * g : 32 * g + K, :], rhs.ap())

            blockmins = data_pool.tile([MBLK, NMB], mybir.dt.float32)

            for s in range(SB):
                w0 = _win_start(s)
                part = work_pool.tile(
                    [MBLK, 4, DIRECT_COLS + 2], mybir.dt.float32, tag="part"
                )
                bfb0 = work_pool.tile(
                    [MBLK, 4, HALF * 512], mybir.dt.bfloat16, tag="bfb0"
                )
                bfb1 = work_pool.tile(
                    [MBLK, 4, HALF * 512], mybir.dt.bfloat16, tag="bfb1"
                )
                for t in range(WIN_TILES):
                    n = w0 + t
                    # two 2-bank PSUM tiles per col (blocks 0-1 and 2-3) so
                    # the pool has 4 slots in flight and consumers split into
                    # shorter units -> less head-of-line blocking
                    ps_a = ps_pool.tile([MBLK, 2, NBLK], mybir.dt.float32, tag="ps")
                    ps_b = ps_pool.tile([MBLK, 2, NBLK], mybir.dt.float32, tag="ps")
                    pshalves = [ps_a, ps_b]
                    for j in range(4):
                        m = 4 * s + j
                        nc.tensor.matmul(
                            pshalves[j // 2][:, j % 2, :],
                            lhs_sb[32 * j : 32 * j + K, m * MBLK : (m + 1) * MBLK],
                            rhs_sb[32 * j : 32 * j + K, n * NBLK : (n + 1) * NBLK],
                            start=True,
                            stop=True,
                            tile_position=(32 * j, 0),
                        )
                    if t in DIRECT_POS:
                        for h in range(2):
                            nc.vector.tensor_reduce(
                                part[:, 2 * h : 2 * h + 2, DIRECT_POS.index(t)],
                                pshalves[h][:],
                                axis=mybir.AxisListType.X,
                                op=mybir.AluOpType.min,
                            )
                    else:
                        c = ACT_POS.index(t)
                        dst = bfb0 if c < HALF else bfb1
                        co = (c % HALF) * 512
                        for h in range(2):
                            nc.scalar.copy(
                                dst[:, 2 * h : 2 * h + 2, co : co + 512],
                                pshalves[h][:],
                            )
                    if t == ACT_POS[HALF - 1]:
                        _half_tree(nc, work_pool, bfb0, part[:, :, DIRECT_COLS])
                    elif t == ACT_POS[-1]:
                        _half_tree(nc, work_pool, bfb1, part[:, :, DIRECT_COLS + 1])
                nc.vector.tensor_reduce(
                    blockmins[:, 4 * s : 4 * s + 4],
                    part[:],
                    axis=mybir.AxisListType.X,
                    op=mybir.AluOpType.min,
                )

            nc.sync.dma_start(out.ap(), blockmins[:])
    return nc


def _split_bf16(v):
    """v (fp32) ~= hi + lo with both bf16; residual is O(2^-18 |v|)."""
    hi = v.astype(BF16)
    lo = (v - hi.astype(np.float32)).astype(BF16)
    return hi, lo


def _prep_core_inputs(Q, R):
    """Build the K=16 lhsT (queries) and rhs (refs) bf16 matrices so that
    lhsT.T @ rhs accumulated in fp32 equals |Q|^2 + |R|^2 - 2 Q.R."""
    Qh, Ql = _split_bf16(Q)  # [N, 3]
    Rh, Rl = _split_bf16(-2.0 * R)  # [N, 3]
    nQh, nQl = _split_bf16((Q * Q).sum(axis=1))  # [N]
    nRh, nRl = _split_bf16((R * R).sum(axis=1))  # [N]
    one = np.ones(N, dtype=BF16)

    L = np.empty([K, N], dtype=BF16)
    L[0:3] = Qh.T
    L[3:6] = Qh.T
    L[6:9] = Ql.T
    L[9:12] = Ql.T
    L[12] = nQh
    L[13] = nQl
    L[14] = one
    L[15] = one

    Rm = np.empty([K, N], dtype=BF16)
    Rm[0:3] = Rh.T
    Rm[3:6] = Rl.T
    Rm[6:9] = Rh.T
    Rm[9:12] = Rl.T
    Rm[12] = one
    Rm[13] = one
    Rm[14] = nRh
    Rm[15] = nRl
    return L, Rm


def _try_axon_reset():
    """The axon-tunneled device sporadically wedges (NRT_EXEC_UNIT_UNRECOVERABLE);
    axon_reset() recovers it."""
    try:
        import ctypes

        import jax

        jax.devices()
        lib = ctypes.CDLL("/opt/axon/libaxon_pjrt.so")
        lib.axon_reset.restype = ctypes.c_int64
        lib.axon_reset()
    except Exception:
        pass


def _task_pairs(gts_X, pred_X):
    for b in range(B):
        yield gts_X[b], pred_X[b]  # each gts point -> nearest pred
        yield pred_X[b], gts_X[b]  # each pred point -> nearest gts


def kernel(gts_X, pred_X, gts_normals=None, **_ignored):
    global LAST_RESULTS
    gts_X = np.asarray(gts_X, dtype=np.float32)
    pred_X = np.asarray(pred_X, dtype=np.float32)
    assert gts_X.shape == (B, N, 3) and pred_X.shape == (B, N, 3)

    in_maps = []
    sorted_pairs = []
    for Qr, Rr in _task_pairs(gts_X, pred_X):
        Qs = np.ascontiguousarray(Qr[np.argsort(Qr[:, 2], kind="stable")])
        Rs = np.ascontiguousarray(Rr[np.argsort(Rr[:, 2], kind="stable")])
        sorted_pairs.append((Qs, Rs))
        L, Rm = _prep_core_inputs(Qs, Rs)
        in_maps.append({"lhs": L, "rhs": Rm})

    nc = _build_bass()
    nc.finalize()
    res = None
    for attempt in range(3):
        try:
            res = run_bass_kernel_spmd(nc, in_maps, core_ids=list(range(8)))
            break
        except Exception:
            if attempt == 2:
                raise
            _try_axon_reset()
    LAST_RESULTS = res

    total = 0.0
    for (Qs, Rs), r in zip(sorted_pairs, res.results):
        mins = r["out"].astype(np.float64)  # [128, 64]; query rank = m*128 + p
        mins = mins.T.reshape(-1)  # rank-ordered per-query windowed mins
        # exactness guard: the true NN can only lie outside the window if the
        # squared z-gap to the window edge is below the windowed min
        s_idx = np.arange(N) // (4 * MBLK)
        w0 = np.array([_win_start(int(s)) for s in range(SB)])[s_idx]
        lo = w0 * NBLK  # first ref rank in window
        hi = lo + WIN_TILES * NBLK  # one past last
        zq = Qs[:, 2].astype(np.float64)
        zr = Rs[:, 2].astype(np.float64)
        gap_lo = np.where(lo > 0, zq - zr[np.maximum(lo - 1, 0)], np.inf)
        gap_hi = np.where(hi < N, zr[np.minimum(hi, N - 1)] - zq, np.inf)
        guard = np.minimum(gap_lo, gap_hi) ** 2
        bad = np.nonzero(mins > guard)[0]
        if len(bad):
            Qb = Qs[bad].astype(np.float64)
            d = ((Qb[:, None, :] - Rs[None, :, :].astype(np.float64)) ** 2).sum(-1)
            mins[bad] = d.min(axis=1)
        total += mins.sum()

    loss = total / (B * N)
    return np.asarray(loss, dtype=np.float32)


# revision 21
# speedup vs baseline: 2.0614x; 1.2456x over previous
"""Chamfer distance (pytorch3d defaults) on 8 Trainium2 NeuronCores.

Problem: gts_X, pred_X: [4, 8192, 3] fp32. loss = mean_b mean_n min_p d(x_bn, y_bp)
                                              + mean_b mean_p min_n d(x_bn, y_bp),
d = squared euclidean distance. gts_normals is unused (reference default path).

Sharding: 8 independent tasks = 4 batches x 2 directions, one per core.
Each core computes per-query min_r d(Q_q, R_r) for its (Q, R) pair of
8192-point clouds; the host sums, guards, and averages.

Device algorithm per core (v3):
- Both clouds are sorted by the z coordinate on the host. Each query
  super-block (4 row blocks of 128 sorted queries) only scans a WINDOW of
  WIN_TILES=9 ref col-tiles (4608 sorted refs) centered on its rank range.
  A query's true nearest neighbor can only be outside the window if the
  squared z-gap to the window edge is smaller than the found min; the host
  verifies that condition per query and recomputes the (rare/none) escapes
  exactly in numpy, so the result is exact for any input.
- d[q, r] = |Q|^2 + |R|^2 - 2 Q.R via ONE K=16 bf16 matmul per (128q x 512r)
  tile using an exact hi/lo bf16 split (bf16 products are exact in fp32, PSUM
  accumulates fp32 => ~fp32 precision).
- Matmuls are packed 4x with tile_position row groups.
- Min-reduction: DIRECT_SET col-tiles are min-reduced straight from PSUM by
  the DVE (1x mode); the rest are ACT-copied PSUM->SBUF with a bf16 downcast
  and folded by a DVE tensor_tensor min tree in 2x bf16 mode.
"""

import sys

sys.path.insert(0, "/opt/trn_rl_repo")

import numpy as np
import ml_dtypes

import concourse.bacc as bacc
import concourse.mybir as mybir
from concourse.tile import TileContext
from concourse.bass_utils import run_bass_kernel_spmd

BF16 = ml_dtypes.bfloat16

B = 4
N = 8192
K = 16  # contraction rows after hi/lo split
MBLK = 128  # queries per row block (PSUM partitions)
NBLK = 512  # refs per matmul (one PSUM bank of fp32)
NMB = N // MBLK  # 64 row blocks
NNB = N // NBLK  # 16 col tiles
SB = NMB // 4  # 16 super-blocks of 4 row blocks

WIN_TILES = 4  # ref col-tiles scanned per super-block
# within-window positions reduced directly from PSUM by the DVE (interleaved
# with ACT-copied positions so the PSUM-slot release chain alternates engines)
DIRECT_POS = (0, 2)
ACT_POS = tuple(t for t in range(WIN_TILES) if t not in DIRECT_POS)
DIRECT_COLS = len(DIRECT_POS)
ACT_COLS = len(ACT_POS)
HALF = ACT_COLS // 2  # two half-trees of HALF cols each

LAST_RESULTS = None  # BassKernelResults of the most recent run (for test.py)


def _win_start(s):
    """First ref col-tile of super-block s's window (rank-based, static)."""
    return min(max(s - WIN_TILES // 2, 0), NNB - WIN_TILES)


def _tt_min(nc, out, a, b):
    nc.vector.tensor_tensor(out, a, b, op=mybir.AluOpType.min)


def _half_tree(nc, work_pool, bfb, part_col):
    """Fold bfb [128, 4, HALF*512] bf16 down to part_col [128, 4, 1] fp32
    via 2x-mode TT mins + one short 1x reduce. szX are per-partition free
    element counts."""
    sz1 = HALF * 512 // 2  # per-block run after level 1
    sz2 = sz1 // 2
    sz3 = sz2 // 2
    sz4 = sz3 // 2
    t1 = work_pool.tile([MBLK, 4, sz1], mybir.dt.bfloat16, tag="t1")
    t2 = work_pool.tile([MBLK, 4, sz2], mybir.dt.bfloat16, tag="t2")
    t3 = work_pool.tile([MBLK, 4, sz3], mybir.dt.bfloat16, tag="t3")
    t4 = work_pool.tile([MBLK, 4, sz4], mybir.dt.bfloat16, tag="t4")
    _tt_min(nc, t1[:], bfb[:, :, 0:sz1], bfb[:, :, sz1 : 2 * sz1])
    _tt_min(nc, t2[:], t1[:, :, 0:sz2], t1[:, :, sz2 : 2 * sz2])
    _tt_min(nc, t3[:], t2[:, :, 0:sz3], t2[:, :, sz3 : 2 * sz3])
    _tt_min(nc, t4[:], t3[:, :, 0:sz4], t3[:, :, sz4 : 2 * sz4])
    nc.vector.tensor_reduce(
        part_col, t4[:], axis=mybir.AxisListType.X, op=mybir.AluOpType.min
    )


def _build_bass():
    nc = bacc.Bacc("TRN2")
    lhs = nc.dram_tensor("lhs", [K, N], mybir.dt.bfloat16, kind="ExternalInput")
    rhs = nc.dram_tensor("rhs", [K, N], mybir.dt.bfloat16, kind="ExternalInput")
    out = nc.dram_tensor("out", [MBLK, NMB], mybir.dt.float32, kind="ExternalOutput")

    with TileContext(nc) as tc:
        with (
            tc.tile_pool(name="data", bufs=1) as data_pool,
            tc.tile_pool(name="work", bufs=3) as work_pool,
            tc.tile_pool(name="ps", bufs=4, space="PSUM") as ps_pool,
        ):
            # operands replicated at partition offsets 0/32/64/96 so four
            # row-group-packed matmuls can run concurrently
            lhs_sb = data_pool.tile([128, N], mybir.dt.bfloat16)
            rhs_sb = data_pool.tile([128, N], mybir.dt.bfloat16)
            for g in range(4):
                nc.sync.dma_start(lhs_sb[32 * g : 32 * g + K, :], lhs.ap())
                nc.sync.dma_start(rhs_sb[32 * g : 32 * g + K, :], rhs.ap())

            blockmins = data_pool.tile([MBLK, NMB], mybir.dt.float32)

            for s in range(SB):
                w0 = _win_start(s)
                part = work_pool.tile(
                    [MBLK, 4, DIRECT_COLS + 2], mybir.dt.float32, tag="part"
                )
                bfb0 = work_pool.tile(
                    [MBLK, 4, HALF * 512], mybir.dt.bfloat16, tag="bfb0"
                )
                bfb1 = work_pool.tile(
                    [MBLK, 4, HALF * 512], mybir.dt.bfloat16, tag="bfb1"
                )
                for t in range(WIN_TILES):
                    n = w0 + t
                    # two 2-bank PSUM tiles per col (blocks 0-1 and 2-3) so
                    # the pool has 4 slots in flight and consumers split into
                    # shorter units -> less head-of-line blocking
                    ps_a = ps_pool.tile([MBLK, 2, NBLK], mybir.dt.float32, tag="ps")
                    ps_b = ps_pool.tile([MBLK, 2, NBLK], mybir.dt.float32, tag="ps")
                    pshalves = [ps_a, ps_b]
                    for j in range(4):
                        m = 4 * s + j
                        nc.tensor.matmul(
                            pshalves[j // 2][:, j % 2, :],
                            lhs_sb[32 * j : 32 * j + K, m * MBLK : (m + 1) * MBLK],
                            rhs_sb[32 * j : 32 * j + K, n * NBLK : (n + 1) * NBLK],
                            start=True,
                            stop=True,
                            tile_position=(32 * j, 0),
                        )
                    if t in DIRECT_POS:
                        for h in range(2):
                            nc.vector.tensor_reduce(
                                part[:, 2 * h : 2 * h + 2, DIRECT_POS.index(t)],
                                pshalves[h][:],
                                axis=mybir.AxisListType.X,
                                op=mybir.AluOpType.min,
                            )
                    else:
                        c = ACT_POS.index(t)
                        dst = bfb0 if c < HALF else bfb1
                        co = (c % HALF) * 512
                        for h in range(2):
                            nc.scalar.copy(
                                dst[:, 2 * h : 2 * h + 2, co : co + 512],
                                pshalves[h][:],
                            )
                    if t == ACT_POS[HALF - 1]:
                        _half_tree(nc, work_pool, bfb0, part[:, :, DIRECT_COLS])
                    elif t == ACT_POS[-1]:
                        _half_tree(nc, work_pool, bfb1, part[:, :, DIRECT_COLS + 1])
                nc.vector.tensor_reduce(
                    blockmins[:, 4 * s : 4 * s + 4],
                    part[:],
                    axis=mybir.AxisListType.X,
                    op=mybir.AluOpType.min,
                )

            nc.sync.dma_start(out.ap(), blockmins[:])
    return nc


def _split_bf16(v):
    """v (fp32) ~= hi + lo with both bf16; residual is O(2^-18 |v|)."""
    hi = v.astype(BF16)
    lo = (v - hi.astype(np.float32)).astype(BF16)
    return hi, lo


def _prep_core_inputs(Q, R):
    """Build the K=16 lhsT (queries) and rhs (refs) bf16 matrices so that
    lhsT.T @ rhs accumulated in fp32 equals |Q|^2 + |R|^2 - 2 Q.R."""
    Qh, Ql = _split_bf16(Q)  # [N, 3]
    Rh, Rl = _split_bf16(-2.0 * R)  # [N, 3]
    nQh, nQl = _split_bf16((Q * Q).sum(axis=1))  # [N]
    nRh, nRl = _split_bf16((R * R).sum(axis=1))  # [N]
    one = np.ones(N, dtype=BF16)

    L = np.empty([K, N], dtype=BF16)
    L[0:3] = Qh.T
    L[3:6] = Qh.T
    L[6:9] = Ql.T
    L[9:12] = Ql.T
    L[12] = nQh
    L[13] = nQl
    L[14] = one
    L[15] = one

    Rm = np.empty([K, N], dtype=BF16)
    Rm[0:3] = Rh.T
    Rm[3:6] = Rl.T
    Rm[6:9] = Rh.T
    Rm[9:12] = Rl.T
    Rm[12] = one
    Rm[13] = one
    Rm[14] = nRh
    Rm[15] = nRl
    return L, Rm


def _try_axon_reset():
    """The axon-tunneled device sporadically wedges (NRT_EXEC_UNIT_UNRECOVERABLE);
    axon_reset() recovers it."""
    try:
        import ctypes

        import jax

        jax.devices()
        lib = ctypes.CDLL("/opt/axon/libaxon_pjrt.so")
        lib.axon_reset.restype = ctypes.c_int64
        lib.axon_reset()
    except Exception:
        pass


def _task_pairs(gts_X, pred_X):
    for b in range(B):
        yield gts_X[b], pred_X[b]  # each gts point -> nearest pred
        yield pred_X[b], gts_X[b]  # each pred point -> nearest gts


def kernel(gts_X, pred_X, gts_normals=None, **_ignored):
    global LAST_RESULTS
    gts_X = np.asarray(gts_X, dtype=np.float32)
    pred_X = np.asarray(pred_X, dtype=np.float32)
    assert gts_X.shape == (B, N, 3) and pred_X.shape == (B, N, 3)

    in_maps = []
    sorted_pairs = []
    for Qr, Rr in _task_pairs(gts_X, pred_X):
        Qs = np.ascontiguousarray(Qr[np.argsort(Qr[:, 2], kind="stable")])
        Rs = np.ascontiguousarray(Rr[np.argsort(Rr[:, 2], kind="stable")])
        sorted_pairs.append((Qs, Rs))
        L, Rm = _prep_core_inputs(Qs, Rs)
        in_maps.append({"lhs": L, "rhs": Rm})

    nc = _build_bass()
    nc.finalize()
    res = None
    for attempt in range(3):
        try:
            res = run_bass_kernel_spmd(nc, in_maps, core_ids=list(range(8)))
            break
        except Exception:
            if attempt == 2:
                raise
            _try_axon_reset()
    LAST_RESULTS = res

    total = 0.0
    for (Qs, Rs), r in zip(sorted_pairs, res.results):
        mins = r["out"].astype(np.float64)  # [128, 64]; query rank = m*128 + p
        mins = mins.T.reshape(-1)  # rank-ordered per-query windowed mins
        # exactness guard: the true NN can only lie outside the window if the
        # squared z-gap to the window edge is below the windowed min
        s_idx = np.arange(N) // (4 * MBLK)
        w0 = np.array([_win_start(int(s)) for s in range(SB)])[s_idx]
        lo = w0 * NBLK  # first ref rank in window
        hi = lo + WIN_TILES * NBLK  # one past last
        zq = Qs[:, 2].astype(np.float64)
        zr = Rs[:, 2].astype(np.float64)
        gap_lo = np.where(lo > 0, zq - zr[np.maximum(lo - 1, 0)], np.inf)
        gap_hi = np.where(hi < N, zr[np.minimum(hi, N - 1)] - zq, np.inf)
        guard = np.minimum(gap_lo, gap_hi) ** 2
        bad = np.nonzero(mins > guard)[0]
        if len(bad):
            Qb = Qs[bad].astype(np.float64)
            d = ((Qb[:, None, :] - Rs[None, :, :].astype(np.float64)) ** 2).sum(-1)
            mins[bad] = d.min(axis=1)
        total += mins.sum()

    loss = total / (B * N)
    return np.asarray(loss, dtype=np.float32)


# revision 25
# speedup vs baseline: 2.2963x; 1.1140x over previous
"""Chamfer distance (pytorch3d defaults) on 8 Trainium2 NeuronCores.

Problem: gts_X, pred_X: [4, 8192, 3] fp32. loss = mean_b mean_n min_p d(x_bn, y_bp)
                                              + mean_b mean_p min_n d(x_bn, y_bp),
d = squared euclidean distance. gts_normals is unused (reference default path).

Sharding: 8 independent tasks = 4 batches x 2 directions, one per core.
Each core computes per-query min_r d(Q_q, R_r) for its (Q, R) pair of
8192-point clouds; the host sums, guards, and averages.

Device algorithm per core (v3):
- Both clouds are sorted by the z coordinate on the host. Each query
  super-block (4 row blocks of 128 sorted queries) only scans a WINDOW of
  WIN_TILES=9 ref col-tiles (4608 sorted refs) centered on its rank range.
  A query's true nearest neighbor can only be outside the window if the
  squared z-gap to the window edge is smaller than the found min; the host
  verifies that condition per query and recomputes the (rare/none) escapes
  exactly in numpy, so the result is exact for any input.
- d[q, r] = |Q|^2 + |R|^2 - 2 Q.R via ONE K=16 bf16 matmul per (128q x 512r)
  tile using an exact hi/lo bf16 split (bf16 products are exact in fp32, PSUM
  accumulates fp32 => ~fp32 precision).
- Matmuls are packed 4x with tile_position row groups.
- Min-reduction: DIRECT_SET col-tiles are min-reduced straight from PSUM by
  the DVE (1x mode); the rest are ACT-copied PSUM->SBUF with a bf16 downcast
  and folded by a DVE tensor_tensor min tree in 2x bf16 mode.
"""

import sys

sys.path.insert(0, "/opt/trn_rl_repo")

import numpy as np
import ml_dtypes

import concourse.bacc as bacc
import concourse.mybir as mybir
from concourse.tile import TileContext
from concourse.bass_utils import run_bass_kernel_spmd

BF16 = ml_dtypes.bfloat16

B = 4
N = 8192
K = 16  # contraction rows after hi/lo split
MBLK = 128  # queries per row block (PSUM partitions)
NBLK = 512  # refs per matmul (one PSUM bank of fp32)
NMB = N // MBLK  # 64 row blocks
NNB = N // NBLK  # 16 col tiles
SB = NMB // 4  # 16 super-blocks of 4 row blocks

WIN_TILES = 4  # ref col-tiles scanned per super-block
# within-window positions reduced directly from PSUM by the DVE (interleaved
# with ACT-copied positions so the PSUM-slot release chain alternates engines)
DIRECT_POS = (1,)
ACT_POS = tuple(t for t in range(WIN_TILES) if t not in DIRECT_POS)
DIRECT_COLS = len(DIRECT_POS)
ACT_COLS = len(ACT_POS)
BF0 = ACT_COLS // 2  # cols folded by the first tree
BF1 = ACT_COLS - BF0  # cols folded by the second tree

LAST_RESULTS = None  # BassKernelResults of the most recent run (for test.py)


def _win_start(s):
    """First ref col-tile of super-block s's window (rank-based, static)."""
    return min(max(s - WIN_TILES // 2, 0), NNB - WIN_TILES)


def _tt_min(nc, out, a, b):
    nc.vector.tensor_tensor(out, a, b, op=mybir.AluOpType.min)


def _half_tree(nc, work_pool, bfb, ncols, part_col):
    """Fold bfb [128, 4, ncols*512] bf16 down to part_col [128, 4, 1] fp32
    via 2x-mode TT mins + one short 1x reduce. szX are per-block element
    counts."""
    sz1 = ncols * 512 // 2  # per-block run after level 1
    sz2 = sz1 // 2
    sz3 = sz2 // 2
    sz4 = sz3 // 2
    t1 = work_pool.tile([MBLK, 4, sz1], mybir.dt.bfloat16, tag="t1")
    t2 = work_pool.tile([MBLK, 4, sz2], mybir.dt.bfloat16, tag="t2")
    t3 = work_pool.tile([MBLK, 4, sz3], mybir.dt.bfloat16, tag="t3")
    t4 = work_pool.tile([MBLK, 4, sz4], mybir.dt.bfloat16, tag="t4")
    _tt_min(nc, t1[:], bfb[:, :, 0:sz1], bfb[:, :, sz1 : 2 * sz1])
    _tt_min(nc, t2[:], t1[:, :, 0:sz2], t1[:, :, sz2 : 2 * sz2])
    _tt_min(nc, t3[:], t2[:, :, 0:sz3], t2[:, :, sz3 : 2 * sz3])
    _tt_min(nc, t4[:], t3[:, :, 0:sz4], t3[:, :, sz4 : 2 * sz4])
    nc.vector.tensor_reduce(
        part_col, t4[:], axis=mybir.AxisListType.X, op=mybir.AluOpType.min
    )


def _build_bass():
    nc = bacc.Bacc("TRN2")
    lhs = nc.dram_tensor("lhs", [K, N], mybir.dt.bfloat16, kind="ExternalInput")
    rhs = nc.dram_tensor("rhs", [K, N], mybir.dt.bfloat16, kind="ExternalInput")
    out = nc.dram_tensor("out", [MBLK, NMB], mybir.dt.float32, kind="ExternalOutput")

    with TileContext(nc) as tc:
        with (
            tc.tile_pool(name="data", bufs=1) as data_pool,
            tc.tile_pool(name="work", bufs=3) as work_pool,
            tc.tile_pool(name="ps", bufs=4, space="PSUM") as ps_pool,
        ):
            # operands replicated at partition offsets 0/32/64/96 so four
            # row-group-packed matmuls can run concurrently
            lhs_sb = data_pool.tile([128, N], mybir.dt.bfloat16)
            rhs_sb = data_pool.tile([128, N], mybir.dt.bfloat16)
            for g in range(4):
                nc.sync.dma_start(lhs_sb[32 * g : 32 * g + K, :], lhs.ap())
                nc.sync.dma_start(rhs_sb[32 * g : 32 * g + K, :], rhs.ap())

            blockmins = data_pool.tile([MBLK, NMB], mybir.dt.float32)

            for s in range(SB):
                w0 = _win_start(s)
                part = work_pool.tile(
                    [MBLK, 4, DIRECT_COLS + 2], mybir.dt.float32, tag="part"
                )
                bfb0 = work_pool.tile(
                    [MBLK, 4, BF0 * 512], mybir.dt.bfloat16, tag="bfb0"
                )
                bfb1 = work_pool.tile(
                    [MBLK, 4, BF1 * 512], mybir.dt.bfloat16, tag="bfb1"
                )
                for t in range(WIN_TILES):
                    n = w0 + t
                    # two 2-bank PSUM tiles per col (blocks 0-1 and 2-3) so
                    # the pool has 4 slots in flight and consumers split into
                    # shorter units -> less head-of-line blocking
                    ps_a = ps_pool.tile([MBLK, 2, NBLK], mybir.dt.float32, tag="ps")
                    ps_b = ps_pool.tile([MBLK, 2, NBLK], mybir.dt.float32, tag="ps")
                    pshalves = [ps_a, ps_b]
                    for j in range(4):
                        m = 4 * s + j
                        nc.tensor.matmul(
                            pshalves[j // 2][:, j % 2, :],
                            lhs_sb[32 * j : 32 * j + K, m * MBLK : (m + 1) * MBLK],
                            rhs_sb[32 * j : 32 * j + K, n * NBLK : (n + 1) * NBLK],
                            start=True,
                            stop=True,
                            tile_position=(32 * j, 0),
                        )
                    if t in DIRECT_POS:
                        for h in range(2):
                            nc.vector.tensor_reduce(
                                part[:, 2 * h : 2 * h + 2, DIRECT_POS.index(t)],
                                pshalves[h][:],
                                axis=mybir.AxisListType.X,
                                op=mybir.AluOpType.min,
                            )
                    else:
                        c = ACT_POS.index(t)
                        dst = bfb0 if c < BF0 else bfb1
                        co = (c if c < BF0 else c - BF0) * 512
                        for h in range(2):
                            nc.scalar.copy(
                                dst[:, 2 * h : 2 * h + 2, co : co + 512],
                                pshalves[h][:],
                            )
                    if t == ACT_POS[BF0 - 1]:
                        _half_tree(nc, work_pool, bfb0, BF0, part[:, :, DIRECT_COLS])
                    elif t == ACT_POS[-1]:
                        _half_tree(
                            nc, work_pool, bfb1, BF1, part[:, :, DIRECT_COLS + 1]
                        )
                nc.vector.tensor_reduce(
                    blockmins[:, 4 * s : 4 * s + 4],
                    part[:],
                    axis=mybir.AxisListType.X,
                    op=mybir.AluOpType.min,
                )

            nc.sync.dma_start(out.ap(), blockmins[:])
    return nc


def _split_bf16(v):
    """v (fp32) ~= hi + lo with both bf16; residual is O(2^-18 |v|)."""
    hi = v.astype(BF16)
    lo = (v - hi.astype(np.float32)).astype(BF16)
    return hi, lo


def _prep_core_inputs(Q, R):
    """Build the K=16 lhsT (queries) and rhs (refs) bf16 matrices so that
    lhsT.T @ rhs accumulated in fp32 equals |Q|^2 + |R|^2 - 2 Q.R."""
    Qh, Ql = _split_bf16(Q)  # [N, 3]
    Rh, Rl = _split_bf16(-2.0 * R)  # [N, 3]
    nQh, nQl = _split_bf16((Q * Q).sum(axis=1))  # [N]
    nRh, nRl = _split_bf16((R * R).sum(axis=1))  # [N]
    one = np.ones(N, dtype=BF16)

    L = np.empty([K, N], dtype=BF16)
    L[0:3] = Qh.T
    L[3:6] = Qh.T
    L[6:9] = Ql.T
    L[9:12] = Ql.T
    L[12] = nQh
    L[13] = nQl
    L[14] = one
    L[15] = one

    Rm = np.empty([K, N], dtype=BF16)
    Rm[0:3] = Rh.T
    Rm[3:6] = Rl.T
    Rm[6:9] = Rh.T
    Rm[9:12] = Rl.T
    Rm[12] = one
    Rm[13] = one
    Rm[14] = nRh
    Rm[15] = nRl
    return L, Rm


def _try_axon_reset():
    """The axon-tunneled device sporadically wedges (NRT_EXEC_UNIT_UNRECOVERABLE);
    axon_reset() recovers it."""
    try:
        import ctypes

        import jax

        jax.devices()
        lib = ctypes.CDLL("/opt/axon/libaxon_pjrt.so")
        lib.axon_reset.restype = ctypes.c_int64
        lib.axon_reset()
    except Exception:
        pass


def _task_pairs(gts_X, pred_X):
    for b in range(B):
        yield gts_X[b], pred_X[b]  # each gts point -> nearest pred
        yield pred_X[b], gts_X[b]  # each pred point -> nearest gts


def kernel(gts_X, pred_X, gts_normals=None, **_ignored):
    global LAST_RESULTS
    gts_X = np.asarray(gts_X, dtype=np.float32)
    pred_X = np.asarray(pred_X, dtype=np.float32)
    assert gts_X.shape == (B, N, 3) and pred_X.shape == (B, N, 3)

    in_maps = []
    sorted_pairs = []
    for Qr, Rr in _task_pairs(gts_X, pred_X):
        Qs = np.ascontiguousarray(Qr[np.argsort(Qr[:, 2], kind="stable")])
        Rs = np.ascontiguousarray(Rr[np.argsort(Rr[:, 2], kind="stable")])
        sorted_pairs.append((Qs, Rs))
        L, Rm = _prep_core_inputs(Qs, Rs)
        in_maps.append({"lhs": L, "rhs": Rm})

    nc = _build_bass()
    nc.finalize()
    res = None
    for attempt in range(3):
        try:
            res = run_bass_kernel_spmd(nc, in_maps, core_ids=list(range(8)))
            break
        except Exception:
            if attempt == 2:
                raise
            _try_axon_reset()
    LAST_RESULTS = res

    total = 0.0
    for (Qs, Rs), r in zip(sorted_pairs, res.results):
        mins = r["out"].astype(np.float64)  # [128, 64]; query rank = m*128 + p
        mins = mins.T.reshape(-1)  # rank-ordered per-query windowed mins
        # exactness guard: the true NN can only lie outside the window if the
        # squared z-gap to the window edge is below the windowed min
        s_idx = np.arange(N) // (4 * MBLK)
        w0 = np.array([_win_start(int(s)) for s in range(SB)])[s_idx]
        lo = w0 * NBLK  # first ref rank in window
        hi = lo + WIN_TILES * NBLK  # one past last
        zq = Qs[:, 2].astype(np.float64)
        zr = Rs[:, 2].astype(np.float64)
        gap_lo = np.where(lo > 0, zq - zr[np.maximum(lo - 1, 0)], np.inf)
        gap_hi = np.where(hi < N, zr[np.minimum(hi, N - 1)] - zq, np.inf)
        guard = np.minimum(gap_lo, gap_hi) ** 2
        bad = np.nonzero(mins > guard)[0]
        if len(bad):
            Qb = Qs[bad].astype(np.float64)
            d = ((Qb[:, None, :] - Rs[None, :, :].astype(np.float64)) ** 2).sum(-1)
            mins[bad] = d.min(axis=1)
        total += mins.sum()

    loss = total / (B * N)
    return np.asarray(loss, dtype=np.float32)


# revision 26
# speedup vs baseline: 2.6589x; 1.1579x over previous
"""Chamfer distance (pytorch3d defaults) on 8 Trainium2 NeuronCores.

Problem: gts_X, pred_X: [4, 8192, 3] fp32. loss = mean_b mean_n min_p d(x_bn, y_bp)
                                              + mean_b mean_p min_n d(x_bn, y_bp),
d = squared euclidean distance. gts_normals is unused (reference default path).

Sharding: 8 independent tasks = 4 batches x 2 directions, one per core.
Each core computes per-query min_r d(Q_q, R_r) for its (Q, R) pair of
8192-point clouds; the host sums, guards, and averages.

Device algorithm per core (v3):
- Both clouds are sorted by the z coordinate on the host. Each query
  super-block (4 row blocks of 128 sorted queries) only scans a WINDOW of
  WIN_TILES=9 ref col-tiles (4608 sorted refs) centered on its rank range.
  A query's true nearest neighbor can only be outside the window if the
  squared z-gap to the window edge is smaller than the found min; the host
  verifies that condition per query and recomputes the (rare/none) escapes
  exactly in numpy, so the result is exact for any input.
- d[q, r] = |Q|^2 + |R|^2 - 2 Q.R via ONE K=16 bf16 matmul per (128q x 512r)
  tile using an exact hi/lo bf16 split (bf16 products are exact in fp32, PSUM
  accumulates fp32 => ~fp32 precision).
- Matmuls are packed 4x with tile_position row groups.
- Min-reduction: DIRECT_SET col-tiles are min-reduced straight from PSUM by
  the DVE (1x mode); the rest are ACT-copied PSUM->SBUF with a bf16 downcast
  and folded by a DVE tensor_tensor min tree in 2x bf16 mode.
"""

import sys

sys.path.insert(0, "/opt/trn_rl_repo")

import numpy as np
import ml_dtypes

import concourse.bacc as bacc
import concourse.mybir as mybir
from concourse.tile import TileContext
from concourse.bass_utils import run_bass_kernel_spmd

BF16 = ml_dtypes.bfloat16

B = 4
N = 8192
K = 16  # contraction rows after hi/lo split
MBLK = 128  # queries per row block (PSUM partitions)
NBLK = 512  # refs per matmul (one PSUM bank of fp32)
NMB = N // MBLK  # 64 row blocks
NNB = N // NBLK  # 16 col tiles
SB = NMB // 4  # 16 super-blocks of 4 row blocks

WIN_TILES = 3  # ref col-tiles scanned per super-block
# within-window positions reduced directly from PSUM by the DVE (interleaved
# with ACT-copied positions so the PSUM-slot release chain alternates engines)
DIRECT_POS = (1,)
ACT_POS = tuple(t for t in range(WIN_TILES) if t not in DIRECT_POS)
DIRECT_COLS = len(DIRECT_POS)
ACT_COLS = len(ACT_POS)
BF0 = ACT_COLS // 2  # cols folded by the first tree
BF1 = ACT_COLS - BF0  # cols folded by the second tree

LAST_RESULTS = None  # BassKernelResults of the most recent run (for test.py)


def _win_start(s):
    """First ref col-tile of super-block s's window (rank-based, static)."""
    return min(max(s - WIN_TILES // 2, 0), NNB - WIN_TILES)


def _tt_min(nc, out, a, b):
    nc.vector.tensor_tensor(out, a, b, op=mybir.AluOpType.min)


def _half_tree(nc, work_pool, bfb, ncols, part_col):
    """Fold bfb [128, 4, ncols*512] bf16 down to part_col [128, 4, 1] fp32
    via 2x-mode TT mins + one short 1x reduce. szX are per-block element
    counts."""
    sz1 = ncols * 512 // 2  # per-block run after level 1
    sz2 = sz1 // 2
    sz3 = sz2 // 2
    sz4 = sz3 // 2
    t1 = work_pool.tile([MBLK, 4, sz1], mybir.dt.bfloat16, tag="t1")
    t2 = work_pool.tile([MBLK, 4, sz2], mybir.dt.bfloat16, tag="t2")
    t3 = work_pool.tile([MBLK, 4, sz3], mybir.dt.bfloat16, tag="t3")
    t4 = work_pool.tile([MBLK, 4, sz4], mybir.dt.bfloat16, tag="t4")
    _tt_min(nc, t1[:], bfb[:, :, 0:sz1], bfb[:, :, sz1 : 2 * sz1])
    _tt_min(nc, t2[:], t1[:, :, 0:sz2], t1[:, :, sz2 : 2 * sz2])
    _tt_min(nc, t3[:], t2[:, :, 0:sz3], t2[:, :, sz3 : 2 * sz3])
    _tt_min(nc, t4[:], t3[:, :, 0:sz4], t3[:, :, sz4 : 2 * sz4])
    nc.vector.tensor_reduce(
        part_col, t4[:], axis=mybir.AxisListType.X, op=mybir.AluOpType.min
    )


def _build_bass():
    nc = bacc.Bacc("TRN2")
    lhs = nc.dram_tensor("lhs", [K, N], mybir.dt.bfloat16, kind="ExternalInput")
    rhs = nc.dram_tensor("rhs", [K, N], mybir.dt.bfloat16, kind="ExternalInput")
    out = nc.dram_tensor("out", [MBLK, NMB], mybir.dt.float32, kind="ExternalOutput")

    with TileContext(nc) as tc:
        with (
            tc.tile_pool(name="data", bufs=1) as data_pool,
            tc.tile_pool(name="work", bufs=3) as work_pool,
            tc.tile_pool(name="ps", bufs=4, space="PSUM") as ps_pool,
        ):
            # operands replicated at partition offsets 0/32/64/96 so four
            # row-group-packed matmuls can run concurrently
            lhs_sb = data_pool.tile([128, N], mybir.dt.bfloat16)
            rhs_sb = data_pool.tile([128, N], mybir.dt.bfloat16)
            for g in range(4):
                nc.sync.dma_start(lhs_sb[32 * g : 32 * g + K, :], lhs.ap())
                nc.sync.dma_start(rhs_sb[32 * g : 32 * g + K, :], rhs.ap())

            blockmins = data_pool.tile([MBLK, NMB], mybir.dt.float32)

            for s in range(SB):
                w0 = _win_start(s)
                part = work_pool.tile(
                    [MBLK, 4, DIRECT_COLS + 2], mybir.dt.float32, tag="part"
                )
                bfb0 = work_pool.tile(
                    [MBLK, 4, BF0 * 512], mybir.dt.bfloat16, tag="bfb0"
                )
                bfb1 = work_pool.tile(
                    [MBLK, 4, BF1 * 512], mybir.dt.bfloat16, tag="bfb1"
                )
                for t in range(WIN_TILES):
                    n = w0 + t
                    # two 2-bank PSUM tiles per col (blocks 0-1 and 2-3) so
                    # the pool has 4 slots in flight and consumers split into
                    # shorter units -> less head-of-line blocking
                    ps_a = ps_pool.tile([MBLK, 2, NBLK], mybir.dt.float32, tag="ps")
                    ps_b = ps_pool.tile([MBLK, 2, NBLK], mybir.dt.float32, tag="ps")
                    pshalves = [ps_a, ps_b]
                    for j in range(4):
                        m = 4 * s + j
                        nc.tensor.matmul(
                            pshalves[j // 2][:, j % 2, :],
                            lhs_sb[32 * j : 32 * j + K, m * MBLK : (m + 1) * MBLK],
                            rhs_sb[32 * j : 32 * j + K, n * NBLK : (n + 1) * NBLK],
                            start=True,
                            stop=True,
                            tile_position=(32 * j, 0),
                        )
                    if t in DIRECT_POS:
                        for h in range(2):
                            nc.vector.tensor_reduce(
                                part[:, 2 * h : 2 * h + 2, DIRECT_POS.index(t)],
                                pshalves[h][:],
                                axis=mybir.AxisListType.X,
                                op=mybir.AluOpType.min,
                            )
                    else:
                        c = ACT_POS.index(t)
                        dst = bfb0 if c < BF0 else bfb1
                        co = (c if c < BF0 else c - BF0) * 512
                        for h in range(2):
                            nc.scalar.copy(
                                dst[:, 2 * h : 2 * h + 2, co : co + 512],
                                pshalves[h][:],
                            )
                    if t == ACT_POS[BF0 - 1]:
                        _half_tree(nc, work_pool, bfb0, BF0, part[:, :, DIRECT_COLS])
                    elif t == ACT_POS[-1]:
                        _half_tree(
                            nc, work_pool, bfb1, BF1, part[:, :, DIRECT_COLS + 1]
                        )
                nc.vector.tensor_reduce(
                    blockmins[:, 4 * s : 4 * s + 4],
                    part[:],
                    axis=mybir.AxisListType.X,
                    op=mybir.AluOpType.min,
                )

            nc.sync.dma_start(out.ap(), blockmins[:])
    return nc


def _split_bf16(v):
    """v (fp32) ~= hi + lo with both bf16; residual is O(2^-18 |v|)."""
    hi = v.astype(BF16)
    lo = (v - hi.astype(np.float32)).astype(BF16)
    return hi, lo


def _prep_core_inputs(Q, R):
    """Build the K=16 lhsT (queries) and rhs (refs) bf16 matrices so that
    lhsT.T @ rhs accumulated in fp32 equals |Q|^2 + |R|^2 - 2 Q.R."""
    Qh, Ql = _split_bf16(Q)  # [N, 3]
    Rh, Rl = _split_bf16(-2.0 * R)  # [N, 3]
    nQh, nQl = _split_bf16((Q * Q).sum(axis=1))  # [N]
    nRh, nRl = _split_bf16((R * R).sum(axis=1))  # [N]
    one = np.ones(N, dtype=BF16)

    L = np.empty([K, N], dtype=BF16)
    L[0:3] = Qh.T
    L[3:6] = Qh.T
    L[6:9] = Ql.T
    L[9:12] = Ql.T
    L[12] = nQh
    L[13] = nQl
    L[14] = one
    L[15] = one

    Rm = np.empty([K, N], dtype=BF16)
    Rm[0:3] = Rh.T
    Rm[3:6] = Rl.T
    Rm[6:9] = Rh.T
    Rm[9:12] = Rl.T
    Rm[12] = one
    Rm[13] = one
    Rm[14] = nRh
    Rm[15] = nRl
    return L, Rm


def _try_axon_reset():
    """The axon-tunneled device sporadically wedges (NRT_EXEC_UNIT_UNRECOVERABLE);
    axon_reset() recovers it."""
    try:
        import ctypes

        import jax

        jax.devices()
        lib = ctypes.CDLL("/opt/axon/libaxon_pjrt.so")
        lib.axon_reset.restype = ctypes.c_int64
        lib.axon_reset()
    except Exception:
        pass


def _task_pairs(gts_X, pred_X):
    for b in range(B):
        yield gts_X[b], pred_X[b]  # each gts point -> nearest pred
        yield pred_X[b], gts_X[b]  # each pred point -> nearest gts


def kernel(gts_X, pred_X, gts_normals=None, **_ignored):
    global LAST_RESULTS
    gts_X = np.asarray(gts_X, dtype=np.float32)
    pred_X = np.asarray(pred_X, dtype=np.float32)
    assert gts_X.shape == (B, N, 3) and pred_X.shape == (B, N, 3)

    in_maps = []
    sorted_pairs = []
    for Qr, Rr in _task_pairs(gts_X, pred_X):
        Qs = np.ascontiguousarray(Qr[np.argsort(Qr[:, 2], kind="stable")])
        Rs = np.ascontiguousarray(Rr[np.argsort(Rr[:, 2], kind="stable")])
        sorted_pairs.append((Qs, Rs))
        L, Rm = _prep_core_inputs(Qs, Rs)
        in_maps.append({"lhs": L, "rhs": Rm})

    nc = _build_bass()
    nc.finalize()
    res = None
    for attempt in range(3):
        try:
            res = run_bass_kernel_spmd(nc, in_maps, core_ids=list(range(8)))
            break
        except Exception:
            if attempt == 2:
                raise
            _try_axon_reset()
    LAST_RESULTS = res

    total = 0.0
    for (Qs, Rs), r in zip(sorted_pairs, res.results):
        mins = r["out"].astype(np.float64)  # [128, 64]; query rank = m*128 + p
        mins = mins.T.reshape(-1)  # rank-ordered per-query windowed mins
        # exactness guard: the true NN can only lie outside the window if the
        # squared z-gap to the window edge is below the windowed min
        s_idx = np.arange(N) // (4 * MBLK)
        w0 = np.array([_win_start(int(s)) for s in range(SB)])[s_idx]
        lo = w0 * NBLK  # first ref rank in window
        hi = lo + WIN_TILES * NBLK  # one past last
        zq = Qs[:, 2].astype(np.float64)
        zr = Rs[:, 2].astype(np.float64)
        gap_lo = np.where(lo > 0, zq - zr[np.maximum(lo - 1, 0)], np.inf)
        gap_hi = np.where(hi < N, zr[np.minimum(hi, N - 1)] - zq, np.inf)
        guard = np.minimum(gap_lo, gap_hi) ** 2
        bad = np.nonzero(mins > guard)[0]
        if len(bad):
            Qb = Qs[bad].astype(np.float64)
            d = ((Qb[:, None, :] - Rs[None, :, :].astype(np.float64)) ** 2).sum(-1)
            mins[bad] = d.min(axis=1)
        total += mins.sum()

    loss = total / (B * N)
    return np.asarray(loss, dtype=np.float32)


# revision 29
# speedup vs baseline: 2.6663x; 1.0028x over previous
"""Chamfer distance (pytorch3d defaults) on 8 Trainium2 NeuronCores.

Problem: gts_X, pred_X: [4, 8192, 3] fp32. loss = mean_b mean_n min_p d(x_bn, y_bp)
                                              + mean_b mean_p min_n d(x_bn, y_bp),
d = squared euclidean distance. gts_normals is unused (reference default path).

Sharding: 8 independent tasks = 4 batches x 2 directions, one per core.
Each core computes per-query min_r d(Q_q, R_r) for its (Q, R) pair of
8192-point clouds; the host sums, guards, and averages.

Device algorithm per core (v3):
- Both clouds are sorted by the z coordinate on the host. Each query
  super-block (4 row blocks of 128 sorted queries) only scans a WINDOW of
  WIN_TILES=9 ref col-tiles (4608 sorted refs) centered on its rank range.
  A query's true nearest neighbor can only be outside the window if the
  squared z-gap to the window edge is smaller than the found min; the host
  verifies that condition per query and recomputes the (rare/none) escapes
  exactly in numpy, so the result is exact for any input.
- d[q, r] = |Q|^2 + |R|^2 - 2 Q.R via ONE K=16 bf16 matmul per (128q x 512r)
  tile using an exact hi/lo bf16 split (bf16 products are exact in fp32, PSUM
  accumulates fp32 => ~fp32 precision).
- Matmuls are packed 4x with tile_position row groups.
- Min-reduction: DIRECT_SET col-tiles are min-reduced straight from PSUM by
  the DVE (1x mode); the rest are ACT-copied PSUM->SBUF with a bf16 downcast
  and folded by a DVE tensor_tensor min tree in 2x bf16 mode.
"""

import sys

sys.path.insert(0, "/opt/trn_rl_repo")

import numpy as np
import ml_dtypes

import concourse.bacc as bacc
import concourse.mybir as mybir
from concourse.tile import TileContext
from concourse.bass_utils import run_bass_kernel_spmd

BF16 = ml_dtypes.bfloat16

B = 4
N = 8192
K = 16  # contraction rows after hi/lo split
MBLK = 128  # queries per row block (PSUM partitions)
NBLK = 512  # refs per matmul (one PSUM bank of fp32)
NMB = N // MBLK  # 64 row blocks
NNB = N // NBLK  # 16 col tiles
SB = NMB // 4  # 16 super-blocks of 4 row blocks

WIN_TILES = 3  # ref col-tiles scanned per super-block
# within-window positions reduced directly from PSUM by the DVE (interleaved
# with ACT-copied positions so the PSUM-slot release chain alternates engines)
DIRECT_POS = (1,)
ACT_POS = tuple(t for t in range(WIN_TILES) if t not in DIRECT_POS)
DIRECT_COLS = len(DIRECT_POS)
ACT_COLS = len(ACT_POS)
BF0 = ACT_COLS // 2  # cols folded by the first tree
BF1 = ACT_COLS - BF0  # cols folded by the second tree

LAST_RESULTS = None  # BassKernelResults of the most recent run (for test.py)


def _win_start(s):
    """First ref col-tile of super-block s's window (rank-based, static)."""
    return min(max(s - WIN_TILES // 2, 0), NNB - WIN_TILES)


def _tt_min(nc, out, a, b):
    nc.vector.tensor_tensor(out, a, b, op=mybir.AluOpType.min)


def _half_tree(nc, work_pool, bfb, ncols, part_col):
    """Fold bfb [128, 4, ncols*512] bf16 down to part_col [128, 4, 1] fp32
    via 2x-mode TT mins + one short 1x reduce. szX are per-block element
    counts."""
    sz1 = ncols * 512 // 2  # per-block run after level 1
    sz2 = sz1 // 2
    sz3 = sz2 // 2
    sz4 = sz3 // 2
    t1 = work_pool.tile([MBLK, 4, sz1], mybir.dt.bfloat16, tag="t1")
    t2 = work_pool.tile([MBLK, 4, sz2], mybir.dt.bfloat16, tag="t2")
    t3 = work_pool.tile([MBLK, 4, sz3], mybir.dt.bfloat16, tag="t3")
    t4 = work_pool.tile([MBLK, 4, sz4], mybir.dt.bfloat16, tag="t4")
    _tt_min(nc, t1[:], bfb[:, :, 0:sz1], bfb[:, :, sz1 : 2 * sz1])
    _tt_min(nc, t2[:], t1[:, :, 0:sz2], t1[:, :, sz2 : 2 * sz2])
    _tt_min(nc, t3[:], t2[:, :, 0:sz3], t2[:, :, sz3 : 2 * sz3])
    _tt_min(nc, t4[:], t3[:, :, 0:sz4], t3[:, :, sz4 : 2 * sz4])
    nc.vector.tensor_reduce(
        part_col, t4[:], axis=mybir.AxisListType.X, op=mybir.AluOpType.min
    )


def _build_bass():
    nc = bacc.Bacc("TRN2")
    lhs = nc.dram_tensor("lhs", [K, N], mybir.dt.bfloat16, kind="ExternalInput")
    rhs = nc.dram_tensor("rhs", [K, N], mybir.dt.bfloat16, kind="ExternalInput")
    out = nc.dram_tensor("out", [MBLK, NMB], mybir.dt.float32, kind="ExternalOutput")

    with TileContext(nc) as tc:
        with (
            tc.tile_pool(name="data", bufs=1) as data_pool,
            tc.tile_pool(name="work", bufs=4) as work_pool,
            tc.tile_pool(name="ps", bufs=4, space="PSUM") as ps_pool,
        ):
            # operands replicated at partition offsets 0/32/64/96 so four
            # row-group-packed matmuls can run concurrently
            lhs_sb = data_pool.tile([128, N], mybir.dt.bfloat16)
            rhs_sb = data_pool.tile([128, N], mybir.dt.bfloat16)
            for g in range(4):
                nc.sync.dma_start(lhs_sb[32 * g : 32 * g + K, :], lhs.ap())
                nc.sync.dma_start(rhs_sb[32 * g : 32 * g + K, :], rhs.ap())

            blockmins = data_pool.tile([MBLK, NMB], mybir.dt.float32)

            for s in range(SB):
                w0 = _win_start(s)
                part = work_pool.tile(
                    [MBLK, 4, DIRECT_COLS + 2], mybir.dt.float32, tag="part"
                )
                bfb0 = work_pool.tile(
                    [MBLK, 4, BF0 * 512], mybir.dt.bfloat16, tag="bfb0"
                )
                bfb1 = work_pool.tile(
                    [MBLK, 4, BF1 * 512], mybir.dt.bfloat16, tag="bfb1"
                )
                for t in range(WIN_TILES):
                    n = w0 + t
                    # two 2-bank PSUM tiles per col (blocks 0-1 and 2-3) so
                    # the pool has 4 slots in flight and consumers split into
                    # shorter units -> less head-of-line blocking
                    ps_a = ps_pool.tile([MBLK, 2, NBLK], mybir.dt.float32, tag="ps")
                    ps_b = ps_pool.tile([MBLK, 2, NBLK], mybir.dt.float32, tag="ps")
                    pshalves = [ps_a, ps_b]
                    for j in range(4):
                        m = 4 * s + j
                        nc.tensor.matmul(
                            pshalves[j // 2][:, j % 2, :],
                            lhs_sb[32 * j : 32 * j + K, m * MBLK : (m + 1) * MBLK],
                            rhs_sb[32 * j : 32 * j + K, n * NBLK : (n + 1) * NBLK],
                            start=True,
                            stop=True,
                            tile_position=(32 * j, 0),
                        )
                    if t in DIRECT_POS:
                        for h in range(2):
                            nc.vector.tensor_reduce(
                                part[:, 2 * h : 2 * h + 2, DIRECT_POS.index(t)],
                                pshalves[h][:],
                                axis=mybir.AxisListType.X,
                                op=mybir.AluOpType.min,
                            )
                    else:
                        c = ACT_POS.index(t)
                        dst = bfb0 if c < BF0 else bfb1
                        co = (c if c < BF0 else c - BF0) * 512
                        for h in range(2):
                            nc.scalar.copy(
                                dst[:, 2 * h : 2 * h + 2, co : co + 512],
                                pshalves[h][:],
                            )
                    if t == ACT_POS[BF0 - 1]:
                        _half_tree(nc, work_pool, bfb0, BF0, part[:, :, DIRECT_COLS])
                    elif t == ACT_POS[-1]:
                        _half_tree(
                            nc, work_pool, bfb1, BF1, part[:, :, DIRECT_COLS + 1]
                        )
                nc.vector.tensor_reduce(
                    blockmins[:, 4 * s : 4 * s + 4],
                    part[:],
                    axis=mybir.AxisListType.X,
                    op=mybir.AluOpType.min,
                )

            nc.sync.dma_start(out.ap(), blockmins[:])
    return nc


def _split_bf16(v):
    """v (fp32) ~= hi + lo with both bf16; residual is O(2^-18 |v|)."""
    hi = v.astype(BF16)
    lo = (v - hi.astype(np.float32)).astype(BF16)
    return hi, lo


def _prep_core_inputs(Q, R):
    """Build the K=16 lhsT (queries) and rhs (refs) bf16 matrices so that
    lhsT.T @ rhs accumulated in fp32 equals |Q|^2 + |R|^2 - 2 Q.R."""
    Qh, Ql = _split_bf16(Q)  # [N, 3]
    Rh, Rl = _split_bf16(-2.0 * R)  # [N, 3]
    nQh, nQl = _split_bf16((Q * Q).sum(axis=1))  # [N]
    nRh, nRl = _split_bf16((R * R).sum(axis=1))  # [N]
    one = np.ones(N, dtype=BF16)

    L = np.empty([K, N], dtype=BF16)
    L[0:3] = Qh.T
    L[3:6] = Qh.T
    L[6:9] = Ql.T
    L[9:12] = Ql.T
    L[12] = nQh
    L[13] = nQl
    L[14] = one
    L[15] = one

    Rm = np.empty([K, N], dtype=BF16)
    Rm[0:3] = Rh.T
    Rm[3:6] = Rl.T
    Rm[6:9] = Rh.T
    Rm[9:12] = Rl.T
    Rm[12] = one
    Rm[13] = one
    Rm[14] = nRh
    Rm[15] = nRl
    return L, Rm


def _try_axon_reset():
    """The axon-tunneled device sporadically wedges (NRT_EXEC_UNIT_UNRECOVERABLE);
    axon_reset() recovers it."""
    try:
        import ctypes

        import jax

        jax.devices()
        lib = ctypes.CDLL("/opt/axon/libaxon_pjrt.so")
        lib.axon_reset.restype = ctypes.c_int64
        lib.axon_reset()
    except Exception:
        pass


def _task_pairs(gts_X, pred_X):
    for b in range(B):
        yield gts_X[b], pred_X[b]  # each gts point -> nearest pred
        yield pred_X[b], gts_X[b]  # each pred point -> nearest gts


def kernel(gts_X, pred_X, gts_normals=None, **_ignored):
    global LAST_RESULTS
    gts_X = np.asarray(gts_X, dtype=np.float32)
    pred_X = np.asarray(pred_X, dtype=np.float32)
    assert gts_X.shape == (B, N, 3) and pred_X.shape == (B, N, 3)

    in_maps = []
    sorted_pairs = []
    for Qr, Rr in _task_pairs(gts_X, pred_X):
        Qs = np.ascontiguousarray(Qr[np.argsort(Qr[:, 2], kind="stable")])
        Rs = np.ascontiguousarray(Rr[np.argsort(Rr[:, 2], kind="stable")])
        sorted_pairs.append((Qs, Rs))
        L, Rm = _prep_core_inputs(Qs, Rs)
        in_maps.append({"lhs": L, "rhs": Rm})

    nc = _build_bass()
    nc.finalize()
    res = None
    for attempt in range(3):
        try:
            res = run_bass_kernel_spmd(nc, in_maps, core_ids=list(range(8)))
            break
        except Exception:
            if attempt == 2:
                raise
            _try_axon_reset()
    LAST_RESULTS = res

    total = 0.0
    for (Qs, Rs), r in zip(sorted_pairs, res.results):
        mins = r["out"].astype(np.float64)  # [128, 64]; query rank = m*128 + p
        mins = mins.T.reshape(-1)  # rank-ordered per-query windowed mins
        # exactness guard: the true NN can only lie outside the window if the
        # squared z-gap to the window edge is below the windowed min
        s_idx = np.arange(N) // (4 * MBLK)
        w0 = np.array([_win_start(int(s)) for s in range(SB)])[s_idx]
        lo = w0 * NBLK  # first ref rank in window
        hi = lo + WIN_TILES * NBLK  # one past last
        zq = Qs[:, 2].astype(np.float64)
        zr = Rs[:, 2].astype(np.float64)
        gap_lo = np.where(lo > 0, zq - zr[np.maximum(lo - 1, 0)], np.inf)
        gap_hi = np.where(hi < N, zr[np.minimum(hi, N - 1)] - zq, np.inf)
        guard = np.minimum(gap_lo, gap_hi) ** 2
        bad = np.nonzero(mins > guard)[0]
        if len(bad):
            Qb = Qs[bad].astype(np.float64)
            d = ((Qb[:, None, :] - Rs[None, :, :].astype(np.float64)) ** 2).sum(-1)
            mins[bad] = d.min(axis=1)
        total += mins.sum()

    loss = total / (B * N)
    return np.asarray(loss, dtype=np.float32)


# revision 31
# speedup vs baseline: 2.7970x; 1.0490x over previous
"""Chamfer distance (pytorch3d defaults) on 8 Trainium2 NeuronCores.

Problem: gts_X, pred_X: [4, 8192, 3] fp32. loss = mean_b mean_n min_p d(x_bn, y_bp)
                                              + mean_b mean_p min_n d(x_bn, y_bp),
d = squared euclidean distance. gts_normals is unused (reference default path).

Sharding: 8 independent tasks = 4 batches x 2 directions, one per core.
Each core computes per-query min_r d(Q_q, R_r) for its (Q, R) pair of
8192-point clouds; the host sums, guards, and averages.

Device algorithm per core:
- Both clouds are sorted by the z coordinate on the host. Each query
  super-block (4 row blocks of 128 sorted queries) only scans a WINDOW of
  WIN_TILES ref col-tiles (WIN_TILES*512 sorted refs) centered on its rank
  range.
  A query's true nearest neighbor can only be outside the window if the
  squared z-gap to the window edge is smaller than the found min; the host
  verifies that condition per query and recomputes the (rare/none) escapes
  exactly in numpy, so the result is exact for any input.
- d[q, r] = |Q|^2 + |R|^2 - 2 Q.R via ONE K=16 bf16 matmul per (128q x 512r)
  tile using an exact hi/lo bf16 split (bf16 products are exact in fp32, PSUM
  accumulates fp32 => ~fp32 precision).
- Matmuls are packed 4x with tile_position row groups.
- Min-reduction: DIRECT_POS col-tiles are min-reduced straight from PSUM by
  the DVE (1x mode); the rest are ACT-copied PSUM->SBUF with a bf16 downcast
  and folded by a DVE tensor_tensor min tree in 2x bf16 mode.
"""

import sys

sys.path.insert(0, "/opt/trn_rl_repo")

import numpy as np
import ml_dtypes

import concourse.bacc as bacc
import concourse.mybir as mybir
from concourse.tile import TileContext
from concourse.bass_utils import run_bass_kernel_spmd

BF16 = ml_dtypes.bfloat16

B = 4
N = 8192
K = 16  # contraction rows after hi/lo split
MBLK = 128  # queries per row block (PSUM partitions)
NBLK = 512  # refs per matmul (one PSUM bank of fp32)
NMB = N // MBLK  # 64 row blocks
NNB = N // NBLK  # 16 col tiles
SB = NMB // 4  # 16 super-blocks of 4 row blocks

WIN_TILES = 3  # ref col-tiles scanned per super-block
# within-window positions reduced directly from PSUM by the DVE (interleaved
# with ACT-copied positions so the PSUM-slot release chain alternates engines)
DIRECT_POS = (1,)
ACT_POS = tuple(t for t in range(WIN_TILES) if t not in DIRECT_POS)
DIRECT_COLS = len(DIRECT_POS)
ACT_COLS = len(ACT_POS)
BF1 = ACT_COLS  # all ACT cols fold through ONE tree (fewer DVE ops)

LAST_RESULTS = None  # BassKernelResults of the most recent run (for test.py)


def _win_start(s):
    """First ref col-tile of super-block s's window (rank-based, static)."""
    return min(max(s - WIN_TILES // 2, 0), NNB - WIN_TILES)


def _tt_min(nc, out, a, b):
    nc.vector.tensor_tensor(out, a, b, op=mybir.AluOpType.min)


def _half_tree(nc, work_pool, bfb, ncols, part_col):
    """Fold bfb [128, 4, ncols*512] bf16 down to part_col [128, 4, 1] fp32
    via 2x-mode TT mins + one short 1x reduce. szX are per-block element
    counts."""
    sz1 = ncols * 512 // 2  # per-block run after level 1
    sz2 = sz1 // 2
    sz3 = sz2 // 2
    sz4 = sz3 // 2
    t1 = work_pool.tile([MBLK, 4, sz1], mybir.dt.bfloat16, tag="t1")
    t2 = work_pool.tile([MBLK, 4, sz2], mybir.dt.bfloat16, tag="t2")
    t3 = work_pool.tile([MBLK, 4, sz3], mybir.dt.bfloat16, tag="t3")
    t4 = work_pool.tile([MBLK, 4, sz4], mybir.dt.bfloat16, tag="t4")
    _tt_min(nc, t1[:], bfb[:, :, 0:sz1], bfb[:, :, sz1 : 2 * sz1])
    _tt_min(nc, t2[:], t1[:, :, 0:sz2], t1[:, :, sz2 : 2 * sz2])
    _tt_min(nc, t3[:], t2[:, :, 0:sz3], t2[:, :, sz3 : 2 * sz3])
    _tt_min(nc, t4[:], t3[:, :, 0:sz4], t3[:, :, sz4 : 2 * sz4])
    nc.vector.tensor_reduce(
        part_col, t4[:], axis=mybir.AxisListType.X, op=mybir.AluOpType.min
    )


def _build_bass():
    nc = bacc.Bacc("TRN2")
    lhs = nc.dram_tensor("lhs", [K, N], mybir.dt.bfloat16, kind="ExternalInput")
    rhs = nc.dram_tensor("rhs", [K, N], mybir.dt.bfloat16, kind="ExternalInput")
    out = nc.dram_tensor("out", [MBLK, NMB], mybir.dt.float32, kind="ExternalOutput")

    with TileContext(nc) as tc:
        with (
            tc.tile_pool(name="data", bufs=1) as data_pool,
            tc.tile_pool(name="work", bufs=4) as work_pool,
            tc.tile_pool(name="ps", bufs=4, space="PSUM") as ps_pool,
        ):
            # operands replicated at partition offsets 0/32/64/96 so four
            # row-group-packed matmuls can run concurrently
            lhs_sb = data_pool.tile([128, N], mybir.dt.bfloat16)
            rhs_sb = data_pool.tile([128, N], mybir.dt.bfloat16)
            for g in range(4):
                nc.sync.dma_start(lhs_sb[32 * g : 32 * g + K, :], lhs.ap())
                nc.sync.dma_start(rhs_sb[32 * g : 32 * g + K, :], rhs.ap())

            blockmins = data_pool.tile([MBLK, NMB], mybir.dt.float32)

            for s in range(SB):
                w0 = _win_start(s)
                part = work_pool.tile(
                    [MBLK, 4, DIRECT_COLS + 1], mybir.dt.float32, tag="part"
                )
                bfb1 = work_pool.tile(
                    [MBLK, 4, BF1 * 512], mybir.dt.bfloat16, tag="bfb1"
                )
                for t in range(WIN_TILES):
                    n = w0 + t
                    # two 2-bank PSUM tiles per col (blocks 0-1 and 2-3) so
                    # the pool has 4 slots in flight and consumers split into
                    # shorter units -> less head-of-line blocking
                    ps_a = ps_pool.tile([MBLK, 2, NBLK], mybir.dt.float32, tag="ps")
                    ps_b = ps_pool.tile([MBLK, 2, NBLK], mybir.dt.float32, tag="ps")
                    pshalves = [ps_a, ps_b]
                    for j in range(4):
                        m = 4 * s + j
                        nc.tensor.matmul(
                            pshalves[j // 2][:, j % 2, :],
                            lhs_sb[32 * j : 32 * j + K, m * MBLK : (m + 1) * MBLK],
                            rhs_sb[32 * j : 32 * j + K, n * NBLK : (n + 1) * NBLK],
                            start=True,
                            stop=True,
                            tile_position=(32 * j, 0),
                        )
                    if t in DIRECT_POS:
                        for h in range(2):
                            nc.vector.tensor_reduce(
                                part[:, 2 * h : 2 * h + 2, DIRECT_POS.index(t)],
                                pshalves[h][:],
                                axis=mybir.AxisListType.X,
                                op=mybir.AluOpType.min,
                            )
                    else:
                        co = ACT_POS.index(t) * 512
                        for h in range(2):
                            nc.scalar.copy(
                                bfb1[:, 2 * h : 2 * h + 2, co : co + 512],
                                pshalves[h][:],
                            )
                    if t == ACT_POS[-1]:
                        _half_tree(
                            nc, work_pool, bfb1, BF1, part[:, :, DIRECT_COLS]
                        )
                nc.vector.tensor_reduce(
                    blockmins[:, 4 * s : 4 * s + 4],
                    part[:],
                    axis=mybir.AxisListType.X,
                    op=mybir.AluOpType.min,
                )

            nc.sync.dma_start(out.ap(), blockmins[:])
    return nc


def _split_bf16(v):
    """v (fp32) ~= hi + lo with both bf16; residual is O(2^-18 |v|)."""
    hi = v.astype(BF16)
    lo = (v - hi.astype(np.float32)).astype(BF16)
    return hi, lo


def _prep_core_inputs(Q, R):
    """Build the K=16 lhsT (queries) and rhs (refs) bf16 matrices so that
    lhsT.T @ rhs accumulated in fp32 equals |Q|^2 + |R|^2 - 2 Q.R."""
    Qh, Ql = _split_bf16(Q)  # [N, 3]
    Rh, Rl = _split_bf16(-2.0 * R)  # [N, 3]
    nQh, nQl = _split_bf16((Q * Q).sum(axis=1))  # [N]
    nRh, nRl = _split_bf16((R * R).sum(axis=1))  # [N]
    one = np.ones(N, dtype=BF16)

    L = np.empty([K, N], dtype=BF16)
    L[0:3] = Qh.T
    L[3:6] = Qh.T
    L[6:9] = Ql.T
    L[9:12] = Ql.T
    L[12] = nQh
    L[13] = nQl
    L[14] = one
    L[15] = one

    Rm = np.empty([K, N], dtype=BF16)
    Rm[0:3] = Rh.T
    Rm[3:6] = Rl.T
    Rm[6:9] = Rh.T
    Rm[9:12] = Rl.T
    Rm[12] = one
    Rm[13] = one
    Rm[14] = nRh
    Rm[15] = nRl
    return L, Rm


def _try_axon_reset():
    """The axon-tunneled device sporadically wedges (NRT_EXEC_UNIT_UNRECOVERABLE);
    axon_reset() recovers it."""
    try:
        import ctypes

        import jax

        jax.devices()
        lib = ctypes.CDLL("/opt/axon/libaxon_pjrt.so")
        lib.axon_reset.restype = ctypes.c_int64
        lib.axon_reset()
    except Exception:
        pass


def _task_pairs(gts_X, pred_X):
    for b in range(B):
        yield gts_X[b], pred_X[b]  # each gts point -> nearest pred
        yield pred_X[b], gts_X[b]  # each pred point -> nearest gts


def kernel(gts_X, pred_X, gts_normals=None, **_ignored):
    global LAST_RESULTS
    gts_X = np.asarray(gts_X, dtype=np.float32)
    pred_X = np.asarray(pred_X, dtype=np.float32)
    assert gts_X.shape == (B, N, 3) and pred_X.shape == (B, N, 3)

    in_maps = []
    sorted_pairs = []
    for Qr, Rr in _task_pairs(gts_X, pred_X):
        Qs = np.ascontiguousarray(Qr[np.argsort(Qr[:, 2], kind="stable")])
        Rs = np.ascontiguousarray(Rr[np.argsort(Rr[:, 2], kind="stable")])
        sorted_pairs.append((Qs, Rs))
        L, Rm = _prep_core_inputs(Qs, Rs)
        in_maps.append({"lhs": L, "rhs": Rm})

    nc = _build_bass()
    nc.finalize()
    res = None
    for attempt in range(3):
        try:
            res = run_bass_kernel_spmd(nc, in_maps, core_ids=list(range(8)))
            break
        except Exception:
            if attempt == 2:
                raise
            _try_axon_reset()
    LAST_RESULTS = res

    total = 0.0
    for (Qs, Rs), r in zip(sorted_pairs, res.results):
        mins = r["out"].astype(np.float64)  # [128, 64]; query rank = m*128 + p
        mins = mins.T.reshape(-1)  # rank-ordered per-query windowed mins
        # exactness guard: the true NN can only lie outside the window if the
        # squared z-gap to the window edge is below the windowed min
        s_idx = np.arange(N) // (4 * MBLK)
        w0 = np.array([_win_start(int(s)) for s in range(SB)])[s_idx]
        lo = w0 * NBLK  # first ref rank in window
        hi = lo + WIN_TILES * NBLK  # one past last
        zq = Qs[:, 2].astype(np.float64)
        zr = Rs[:, 2].astype(np.float64)
        gap_lo = np.where(lo > 0, zq - zr[np.maximum(lo - 1, 0)], np.inf)
        gap_hi = np.where(hi < N, zr[np.minimum(hi, N - 1)] - zq, np.inf)
        guard = np.minimum(gap_lo, gap_hi) ** 2
        bad = np.nonzero(mins > guard)[0]
        if len(bad):
            Qb = Qs[bad].astype(np.float64)
            d = ((Qb[:, None, :] - Rs[None, :, :].astype(np.float64)) ** 2).sum(-1)
            mins[bad] = d.min(axis=1)
        total += mins.sum()

    loss = total / (B * N)
    return np.asarray(loss, dtype=np.float32)
